# revision 1
# baseline (speedup 1.0000x reference)
"""Trainium2 Bass kernel for a cross-attention + adaLN-modulated-LN + linear block.

Sharding: 8 cores = 4 batches (B) x 2 token-halves of S=4096.  No collectives:
each core recomputes the (small) kv projection for its batch and processes all
16 attention heads for its 2048 tokens, then LN/modulation/final-linear for its
8 frames.  Host slices inputs per core and reassembles the output.

Device layout is feature-major ("transposed"): activations live as [C, tok]
tiles so every matmul contracts over the SBUF partition dim.  x and v are
transposed on the PE with identity matmuls (float32r, batched 4-group PSUM
evictions).  Softmax runs on scoresT [key, tok] tiles; the softmax denominator
rides along the attention-output matmul as an all-ones lhsT column.  The final
LayerNorm modulation is folded into the output matmul algebraically:
  y = rstd (.) (x1T @ (wlin (.) s1))  -  (mu rstd) (.) ws1  +  (sh @ wlin) + blin
so no elementwise pass over [C, tok] is needed after the projection.

Weights are pre-blocked on the host ([cout-tile, cin-tile, 128, 128]) so each
weight DMA is a single large descriptor.
"""

import sys

for _p in ("/opt/trn_rl_repo", "/opt/pypackages"):
    if _p not in sys.path:
        sys.path.append(_p)

import numpy as np

import concourse.bacc as bacc
import concourse.tile as tile
from concourse import mybir
from concourse.bass_utils import run_bass_kernel_spmd
from concourse.masks import make_identity

FP = mybir.dt.float32
FPR = mybir.dt.float32r
AF = mybir.ActivationFunctionType
OP = mybir.AluOpType


def _r(ap):
    """View an fp32 AP as float32r for full-rate PE matmuls (same bits)."""
    return ap.bitcast(FPR)


# Problem sizes (hardcoded per spec).
B = 4
S = 4096
C = 1024
N2 = 512
H = 16
D = 64
T = 16
NT = 256          # tokens per frame
OUTD = 32

STOK = S // 2     # tokens per core
F = 8             # frames per core
G = C // 128      # 8 channel groups
TB = 512          # token block (matmul N)
NTB = STOK // TB  # 4
KB = N2 // 128    # 4 key blocks
SCALE = D ** -0.5
EPS = 1e-6
P = 128
SEG = 192         # vv cols per head-pair segment


def _body(nc, tc, io):
    with nc.allow_low_precision("fp32r-rounded matmul operands"):
        _body_inner(nc, tc, io)


def _body_inner(nc, tc, io):
    x, v, tvec, cmat = io["x_sl"], io["v_b"], io["t_b"], io["c_sl"]
    wq_b, bq = io["wq_b"], io["bq"]
    wkvk_b, wkvv, bkv = io["wkvk_b"], io["wkvv"], io["bkv"]
    wproj_b, bproj = io["wproj_b"], io["bproj"]
    wada_b, bada = io["wada_b"], io["bada"]
    wlin, blin = io["wlin"], io["blin"]
    yT = io["yT"]

    with (
        tc.tile_pool(name="consts", bufs=1) as consts,
        tc.tile_pool(name="xT", bufs=1) as xTp,
        tc.tile_pool(name="qa", bufs=1) as qap,
        tc.tile_pool(name="kT", bufs=1) as kTp,
        tc.tile_pool(name="wp", bufs=3) as wp,
    ):
        # ---- constants / small inputs ----
        scratch = consts.tile([P, P], FP, tag="scratch")
        make_identity(nc, scratch)
        ident = consts.tile([P, P], FP, tag="ident")
        nc.vector.tensor_copy(out=_r(ident), in_=scratch)
        ones_t = consts.tile([P, P], FP, tag="ones")
        nc.vector.tensor_scalar(
            _r(ones_t), scratch, 0.0, 1.0, op0=OP.mult, op1=OP.add
        )
        eps_t = consts.tile([P, 1], FP, tag="eps")
        nc.vector.memset(eps_t, EPS)

        bq_t = consts.tile([P, G], FP, tag="bq")
        nc.sync.dma_start(out=bq_t, in_=bq.ap().rearrange("(g p) -> p g", p=P))
        bkvk_t = consts.tile([P, G], FP, tag="bkvk")
        nc.sync.dma_start(
            out=bkvk_t, in_=bkv.ap()[0:C].rearrange("(g p) -> p g", p=P)
        )
        bproj_t = consts.tile([P, G], FP, tag="bproj")
        nc.sync.dma_start(out=bproj_t, in_=bproj.ap().rearrange("(g p) -> p g", p=P))
        bada_t = consts.tile([P, 16], FP, tag="bada")
        nc.sync.dma_start(out=bada_t, in_=bada.ap().rearrange("(g p) -> p g", p=P))
        blin_row = consts.tile([1, OUTD], FP, tag="blin")
        nc.sync.dma_start(
            out=blin_row, in_=blin.ap().rearrange("(one o) -> one o", one=1)
        )
        t_t = consts.tile([P, G], FP, tag="tvec")
        nc.sync.dma_start(out=t_t, in_=tvec.ap().rearrange("(g p) -> p g", p=P))
        wlin_sb = consts.tile([P, G, OUTD], FP, tag="wlin")
        nc.sync.dma_start(
            out=_r(wlin_sb), in_=_r(wlin.ap().rearrange("(ci p) o -> p ci o", p=P))
        )
        silu_t = consts.tile([P, G, F], FP, tag="silu")
        ada_t = consts.tile([P, 16, F], FP, tag="ada")

        # ---- persistent activation buffers ----
        xT_t = xTp.tile([P, G, STOK], FP, tag="xT")       # x then x1 (feature-major)
        qa_t = qap.tile([P, G, STOK], FP, tag="qa")       # q then attn-out
        kt_t = kTp.tile([P, G, N2], FP, tag="kT")         # k (feature-major)

        # vv: key-major value matrix. Per head-pair g, a SEG=192-col segment:
        #   [0:64]    even-head data (lhsT cols 0..64 incl. ones -> denom row 64)
        #   [64]      ones column (serves both heads)
        #   [65:128]  junk (feeds only unread rows of the odd window)
        #   [128:192] odd-head data (128-wide lhsT window from col 64:
        #             ones lands at window col 0 -> denom row 0,
        #             data at window cols 64..127 -> ao rows 64..127)
        with tc.tile_pool(name="vv", bufs=1) as vvp:
            vv = [
                vvp.tile([P, 8 * SEG], FP, name=f"vv{kb}", tag=f"vv{kb}")
                for kb in range(KB)
            ]
            for kb in range(KB):
                # only the per-segment ones-columns need initializing: data cols
                # are written by the eviction copies, junk cols feed only unread
                # output rows of the odd-head lhsT windows.
                ones_cols = vv[kb].rearrange("p (a r) -> p a r", r=SEG)[:, :, 64:65]
                nc.vector.tensor_scalar(
                    _r(ones_cols), scratch[:, 0:8].rearrange("p (a u) -> p a u", u=1), 0.0, 1.0,
                    op0=OP.mult, op1=OP.add,
                )

            with tc.tile_pool(name="psA", bufs=8, space="PSUM") as psA:
                with (
                    tc.tile_pool(name="vT", bufs=1) as vTp,
                    tc.tile_pool(name="ld", bufs=4) as ldp,
                ):
                    vT_t = vTp.tile([P, G, N2], FP, tag="vT")

                    # ---- transpose v ----
                    for kt in range(KB):
                        for g4 in range(2):
                            v_nat = ldp.tile([P, TB], FP, name="vn", tag="ld")
                            nc.scalar.dma_start(
                                out=_r(v_nat),
                                in_=_r(v[kt * P : (kt + 1) * P, g4 * TB : (g4 + 1) * TB]),
                            )
                            pt = psA.tile([P, 4 * P], FP, name="ptv", tag="ps")
                            for j in range(4):
                                nc.tensor.transpose(
                                    _r(pt[:, j * P : (j + 1) * P]),
                                    _r(v_nat[:, j * P : (j + 1) * P]),
                                    _r(ident),
                                )
                            nc.any.tensor_copy(
                                out=_r(vT_t[:, g4 * 4 : g4 * 4 + 4, kt * P : (kt + 1) * P]),
                                in_=pt.rearrange("p (a c) -> p a c", c=P),
                            )

                    # ---- vv build (key-major), bias folded via ones-row MM ----
                    bkvv_row = ldp.tile([1, C], FP, name="bkvv", tag="misc", bufs=1)
                    nc.sync.dma_start(
                        out=_r(bkvv_row),
                        in_=_r(bkv.ap()[C : 2 * C].rearrange("(one n) -> one n", one=1)),
                    )
                    for half in range(2):
                        pss = [
                            psA.tile([P, TB], FP, name="psv", tag="ps")
                            for _ in range(KB)
                        ]
                        for ci in range(G):
                            wv = wp.tile([P, TB], FP, name="wv", tag="w")
                            nc.gpsimd.dma_start(
                                out=_r(wv),
                                in_=_r(wkvv[
                                    ci * P : (ci + 1) * P,
                                    half * TB : (half + 1) * TB,
                                ]),
                            )
                            for kb in range(KB):
                                nc.tensor.matmul(
                                    pss[kb],
                                    lhsT=_r(vT_t[:, ci, kb * P : (kb + 1) * P]),
                                    rhs=_r(wv),
                                    start=(ci == 0),
                                    stop=False,
                                )
                        for kb in range(KB):
                            nc.tensor.matmul(
                                pss[kb],
                                lhsT=_r(ones_t[0:1, 0:P]),
                                rhs=_r(bkvv_row[0:1, half * TB : (half + 1) * TB]),
                                start=False,
                                stop=True,
                            )
                            vvr = vv[kb].rearrange("p (a r) -> p a r", r=SEG)
                            src = pss[kb].rearrange("p (a q j) -> p a q j", q=2, j=64)
                            gs = slice(half * 4, half * 4 + 4)
                            nc.vector.tensor_copy(
                                out=_r(vvr[:, gs, 0:64]), in_=src[:, :, 0, :]
                            )
                            nc.vector.tensor_copy(
                                out=_r(vvr[:, gs, 128:192]), in_=src[:, :, 1, :]
                            )

                    # ---- kT (all groups; frees vT afterwards) ----
                    for g in range(G):
                        wt0 = wp.tile([P, 4, P], FP, name="wtk0", tag="w")
                        wt1 = wp.tile([P, 4, P], FP, name="wtk1", tag="w")
                        nc.scalar.dma_start(
                            out=_r(wt0), in_=_r(wkvk_b[g, 0:4].rearrange("ci p c -> p ci c"))
                        )
                        nc.gpsimd.dma_start(
                            out=_r(wt1), in_=_r(wkvk_b[g, 4:8].rearrange("ci p c -> p ci c"))
                        )
                        wts = (wt0, wt1)
                        psk = psA.tile([P, N2], FP, name="psk", tag="ps")
                        for ci in range(G):
                            nc.tensor.matmul(
                                psk,
                                lhsT=_r(wts[ci // 4][:, ci % 4, :]),
                                rhs=_r(vT_t[:, ci, :]),
                                start=(ci == 0),
                                stop=(ci == G - 1),
                            )
                        nc.vector.tensor_scalar_add(
                            _r(kt_t[:, g, :]), psk, bkvk_t[:, g : g + 1]
                        )

                    # ---- transpose x ----
                    for tt in range(STOK // P):
                        for g4 in range(2):
                            x_nat = ldp.tile([P, TB], FP, name="xn", tag="ld")
                            dma_eng = (nc.sync, nc.scalar, nc.gpsimd)[(2 * tt + g4) % 3]
                            dma_eng.dma_start(
                                out=_r(x_nat),
                                in_=_r(x[tt * P : (tt + 1) * P, g4 * TB : (g4 + 1) * TB]),
                            )
                            pt = psA.tile([P, 4 * P], FP, name="ptx", tag="ps")
                            for j in range(4):
                                nc.tensor.transpose(
                                    _r(pt[:, j * P : (j + 1) * P]),
                                    _r(x_nat[:, j * P : (j + 1) * P]),
                                    _r(ident),
                                )
                            nc.any.tensor_copy(
                                out=_r(xT_t[:, g4 * 4 : g4 * 4 + 4, tt * P : (tt + 1) * P]),
                                in_=pt.rearrange("p (a c) -> p a c", c=P),
                            )

                    # ---- adaLN: silu(t + c) @ wada + bada ----
                    c_nat = ldp.tile([F, C], FP, name="cnat", tag="misc", bufs=1)
                    nc.sync.dma_start(out=_r(c_nat), in_=_r(cmat[:, :]))
                    for g in range(G):
                        pt = psA.tile([P, F], FP, name="ptc", tag="ps")
                        nc.tensor.transpose(
                            _r(pt), _r(c_nat[:, g * P : (g + 1) * P]), _r(ident[0:F, 0:F])
                        )
                        nc.scalar.activation(
                            out=_r(silu_t[:, g, :]),
                            in_=pt,
                            func=AF.Silu,
                            bias=t_t[:, g : g + 1],
                            scale=1.0,
                        )
                    for ct in range(16):
                        wt0 = wp.tile([P, 4, P], FP, name="wta0", tag="w")
                        wt1 = wp.tile([P, 4, P], FP, name="wta1", tag="w")
                        nc.sync.dma_start(
                            out=_r(wt0), in_=_r(wada_b[ct, 0:4].rearrange("ci p c -> p ci c"))
                        )
                        nc.sync.dma_start(
                            out=_r(wt1), in_=_r(wada_b[ct, 4:8].rearrange("ci p c -> p ci c"))
                        )
                        wts = (wt0, wt1)
                        pa = psA.tile([P, F], FP, name="pta", tag="ps")
                        for ci in range(G):
                            nc.tensor.matmul(
                                pa,
                                lhsT=_r(wts[ci // 4][:, ci % 4, :]),
                                rhs=_r(silu_t[:, ci, :]),
                                start=(ci == 0),
                                stop=(ci == G - 1),
                            )
                        if ct < 8:
                            # ada cols 0..1023 = shift -> ct 0..7
                            nc.vector.tensor_scalar_add(
                                _r(ada_t[:, ct, :]), pa, bada_t[:, ct : ct + 1]
                            )
                        else:
                            # ada cols 1024..2047 = sc -> ct 8..15 hold (1 + sc)
                            nc.vector.tensor_scalar(
                                _r(ada_t[:, ct, :]),
                                pa,
                                bada_t[:, ct : ct + 1],
                                1.0,
                                op0=OP.add,
                                op1=OP.add,
                            )


                # ---- q projection ----
                for g in range(G):
                    wt0 = wp.tile([P, 4, P], FP, name="wtq0", tag="w")
                    wt1 = wp.tile([P, 4, P], FP, name="wtq1", tag="w")
                    nc.sync.dma_start(
                        out=_r(wt0), in_=_r(wq_b[g, 0:4].rearrange("ci p c -> p ci c"))
                    )
                    nc.sync.dma_start(
                        out=_r(wt1), in_=_r(wq_b[g, 4:8].rearrange("ci p c -> p ci c"))
                    )
                    wts = (wt0, wt1)
                    pst = [
                        psA.tile([P, TB], FP, name="psq", tag="ps")
                        for _ in range(NTB)
                    ]
                    for ci in range(G):
                        for tb in range(NTB):
                            nc.tensor.matmul(
                                pst[tb],
                                lhsT=_r(wts[ci // 4][:, ci % 4, :]),
                                rhs=_r(xT_t[:, ci, tb * TB : (tb + 1) * TB]),
                                start=(ci == 0),
                                stop=(ci == G - 1),
                            )
                    for tb in range(NTB):
                        nc.vector.tensor_scalar_add(
                            _r(qa_t[:, g, tb * TB : (tb + 1) * TB]),
                            pst[tb],
                            bq_t[:, g : g + 1],
                        )

            # ---- attention (per head; pipelined over (h, tb)) ----
            # psC (proj psums) opens alongside the attention pools so the
            # projection chains can start as soon as their token block's
            # attention wave completes.
            psC_cm = tc.tile_pool(name="psC", bufs=2, space="PSUM")
            psC = psC_cm.__enter__()
            with (
                tc.tile_pool(name="sc", bufs=3, space="PSUM") as scp,
                tc.tile_pool(name="ao", bufs=2, space="PSUM") as aop,
                tc.tile_pool(name="bc", bufs=1, space="PSUM") as bcpp,
                tc.tile_pool(name="exp", bufs=4) as expp,
                tc.tile_pool(name="dn", bufs=3) as dnp,
            ):
                for tb in range(NTB):
                    tbs = slice(tb * TB, (tb + 1) * TB)
                    for g in range(G):
                        for half in range(2):
                            h = 2 * g + half
                            r0 = half * 64
                            dr = 64 - 64 * half  # denom row: 64 (even), 0 (odd)
                            if half == 0:
                                lhs_lo, lhs_hi = g * SEG, g * SEG + 65
                            else:
                                lhs_lo, lhs_hi = g * SEG + 64, g * SEG + SEG
                            if half == 0:
                                ao_ps = aop.tile([65, TB], FP, name="aoe", tag="ao")
                                ao_rows = slice(0, 64)
                            else:
                                ao_ps = aop.tile([P, TB], FP, name="aoo", tag="ao")
                                ao_rows = slice(64, 128)
                            for kb in range(KB):
                                sc_ps = scp.tile([P, TB], FP, name="scs", tag="sc")
                                nc.tensor.matmul(
                                    sc_ps,
                                    lhsT=_r(
                                        kt_t[r0 : r0 + 64, g, kb * P : (kb + 1) * P]
                                    ),
                                    rhs=_r(qa_t[r0 : r0 + 64, g, tbs]),
                                    start=True,
                                    stop=True,
                                )
                                ex = expp.tile([P, TB], FP, tag="e")
                                nc.scalar.activation(
                                    out=_r(ex), in_=sc_ps, func=AF.Exp, scale=SCALE
                                )
                                nc.tensor.matmul(
                                    ao_ps,
                                    lhsT=_r(vv[kb][:, lhs_lo:lhs_hi]),
                                    rhs=_r(ex),
                                    start=(kb == 0),
                                    stop=(kb == KB - 1),
                                )
                            # softmax normalization via the ones-column row
                            dnb = dnp.tile([P, TB], FP, tag="dn")
                            nc.vector.reciprocal(
                                out=_r(dnb[dr : dr + 1, :]), in_=ao_ps[dr : dr + 1, :]
                            )
                            bc_ps = bcpp.tile([P, TB], FP, name="bcp", tag="bc")
                            nc.tensor.matmul(
                                bc_ps,
                                lhsT=_r(ones_t[dr : dr + 1, :]),
                                rhs=_r(dnb[dr : dr + 1, :]),
                                start=True,
                                stop=True,
                            )
                            nc.vector.tensor_copy(
                                out=_r(dnb[r0 : r0 + 64, :]), in_=bc_ps[r0 : r0 + 64, :]
                            )
                            nc.vector.tensor_mul(
                                _r(qa_t[r0 : r0 + 64, g, tbs]),
                                ao_ps[ao_rows, :],
                                dnb[r0 : r0 + 64, :],
                            )

        # ---- proj + residual, then LN + folded modulation + final linear ----
        for tb in range(NTB):
            tbs = slice(tb * TB, (tb + 1) * TB)
            for g in range(G):
                wt0 = wp.tile([P, 4, P], FP, name="wtp0", tag="w")
                wt1 = wp.tile([P, 4, P], FP, name="wtp1", tag="w")
                nc.sync.dma_start(
                    out=_r(wt0), in_=_r(wproj_b[g, 0:4].rearrange("ci p c -> p ci c"))
                )
                nc.sync.dma_start(
                    out=_r(wt1), in_=_r(wproj_b[g, 4:8].rearrange("ci p c -> p ci c"))
                )
                wts = (wt0, wt1)
                pst = psC.tile([P, TB], FP, name="psp", tag="ps")
                for ci in range(G):
                    nc.tensor.matmul(
                        pst,
                        lhsT=_r(wts[ci // 4][:, ci % 4, :]),
                        rhs=_r(qa_t[:, ci, tbs]),
                        start=(ci == 0),
                        stop=(ci == G - 1),
                    )
                nc.vector.scalar_tensor_tensor(
                    out=_r(xT_t[:, g, tbs]),
                    in0=pst,
                    scalar=bproj_t[:, g : g + 1],
                    in1=xT_t[:, g, tbs],
                    op0=OP.add,
                    op1=OP.add,
                )
        with (
            tc.tile_pool(name="psD", bufs=6, space="PSUM") as psD,
            tc.tile_pool(name="tmp", bufs=3) as tmpp,
            tc.tile_pool(name="st", bufs=6) as stp,
            tc.tile_pool(name="w1", bufs=2) as w1p,
            tc.tile_pool(name="rows", bufs=4) as rowp,
            tc.tile_pool(name="nrm", bufs=2) as nrmp,
            tc.tile_pool(name="yo", bufs=2) as yop,
        ):
            for tb in range(NTB):
                tbs = slice(tb * TB, (tb + 1) * TB)
                ln_a = psD.tile([1, TB], FP, name="lna", tag="ps")
                ln_b = psD.tile([1, TB], FP, name="lnb", tag="ps")
                for g in range(G):
                    sqt = tmpp.tile([P, TB], FP, tag="tmp")
                    nc.scalar.activation(
                        out=_r(sqt), in_=xT_t[:, g, tbs], func=AF.Square
                    )
                    nc.tensor.matmul(
                        ln_a,
                        lhsT=_r(ones_t[:, 0:1]),
                        rhs=_r(xT_t[:, g, tbs]),
                        start=(g == 0),
                        stop=(g == G - 1),
                    )
                    nc.tensor.matmul(
                        ln_b,
                        lhsT=_r(ones_t[:, 0:1]),
                        rhs=_r(sqt),
                        start=(g == 0),
                        stop=(g == G - 1),
                    )
                mu = stp.tile([1, TB], FP, name="mu", tag="st")
                std = stp.tile([1, TB], FP, name="std", tag="st")
                rst = stp.tile([1, TB], FP, name="rst", tag="st")
                nc.vector.tensor_scalar_mul(_r(mu), ln_a, 1.0 / C)
                nc.vector.tensor_mul(_r(std), mu, mu)
                nc.vector.scalar_tensor_tensor(
                    out=_r(std),
                    in0=ln_b,
                    scalar=1.0 / C,
                    in1=std,
                    op0=OP.mult,
                    op1=OP.subtract,
                )
                nc.scalar.activation(
                    out=_r(std), in_=std, func=AF.Sqrt, bias=eps_t[0:1, :], scale=1.0
                )
                nc.vector.reciprocal(_r(rst), std)
                bc32_ps = psD.tile([32, TB], FP, name="bc32", tag="ps")
                nc.tensor.matmul(
                    bc32_ps,
                    lhsT=_r(ones_t[0:1, 0:32]),
                    rhs=_r(rst),
                    start=True,
                    stop=True,
                )
                bc32 = nrmp.tile([32, TB], FP, tag="nrm")
                nc.scalar.copy(out=bc32, in_=bc32_ps)
                for f2 in range(2):
                    f = tb * 2 + f2
                    fcs = slice(f2 * NT, (f2 + 1) * NT)  # cols within tb
                    gcs = slice(tb * TB + f2 * NT, tb * TB + (f2 + 1) * NT)
                    w1 = w1p.tile([P, G, OUTD], FP, tag="w1")
                    for g in range(G):
                        nc.vector.tensor_scalar_mul(
                            _r(w1[:, g, :]),
                            wlin_sb[:, g, :],
                            ada_t[:, 8 + g, f : f + 1],
                        )
                    ws1_ps = psD.tile([1, OUTD], FP, name="ws1", tag="ps")
                    c2_ps = psD.tile([1, OUTD], FP, name="c2", tag="ps")
                    for g in range(G):
                        nc.tensor.matmul(
                            ws1_ps,
                            lhsT=_r(ada_t[:, 8 + g, f : f + 1]),
                            rhs=_r(wlin_sb[:, g, :]),
                            start=(g == 0),
                            stop=(g == G - 1),
                        )
                        nc.tensor.matmul(
                            c2_ps,
                            lhsT=_r(ada_t[:, g, f : f + 1]),
                            rhs=_r(wlin_sb[:, g, :]),
                            start=(g == 0),
                            stop=(g == G - 1),
                        )
                    ws1n = rowp.tile([1, OUTD], FP, name="ws1n", tag="rows")
                    c2b = rowp.tile([1, OUTD], FP, name="c2b", tag="rows")
                    nc.vector.tensor_scalar_mul(_r(ws1n), ws1_ps, -1.0)
                    nc.vector.tensor_tensor(_r(c2b), c2_ps, blin_row, OP.add)
                    y_ps = psD.tile([OUTD, NT], FP, name="yps", tag="ps")
                    for g in range(G):
                        nc.tensor.matmul(
                            y_ps,
                            lhsT=_r(w1[:, g, :]),
                            rhs=_r(xT_t[:, g, gcs]),
                            start=(g == 0),
                            stop=False,
                        )
                    nc.tensor.matmul(
                        y_ps,
                        lhsT=_r(ws1n),
                        rhs=_r(mu[0:1, fcs]),
                        start=False,
                        stop=False,
                    )
                    nc.tensor.matmul(
                        y_ps,
                        lhsT=_r(c2b),
                        rhs=_r(std[0:1, fcs]),
                        start=False,
                        stop=True,
                    )
                    yt = yop.tile([OUTD, NT], FP, tag="y")
                    nc.vector.tensor_mul(yt, y_ps, bc32[:, fcs])
                    nc.sync.dma_start(out=yT[:, gcs], in_=yt)

        psC_cm.__exit__(None, None, None)

def declare_io(nc):
    return {
        "x_sl": nc.dram_tensor("x_sl", [STOK, C], FP, kind="ExternalInput"),
        "v_b": nc.dram_tensor("v_b", [N2, C], FP, kind="ExternalInput"),
        "t_b": nc.dram_tensor("t_b", [C], FP, kind="ExternalInput"),
        "c_sl": nc.dram_tensor("c_sl", [F, C], FP, kind="ExternalInput"),
        "wq_b": nc.dram_tensor("wq_b", [G, G, P, P], FP, kind="ExternalInput"),
        "bq": nc.dram_tensor("bq", [C], FP, kind="ExternalInput"),
        "wkvk_b": nc.dram_tensor("wkvk_b", [G, G, P, P], FP, kind="ExternalInput"),
        "wkvv": nc.dram_tensor("wkvv", [C, C], FP, kind="ExternalInput"),
        "bkv": nc.dram_tensor("bkv", [2 * C], FP, kind="ExternalInput"),
        "wproj_b": nc.dram_tensor("wproj_b", [G, G, P, P], FP, kind="ExternalInput"),
        "bproj": nc.dram_tensor("bproj", [C], FP, kind="ExternalInput"),
        "wada_b": nc.dram_tensor("wada_b", [16, G, P, P], FP, kind="ExternalInput"),
        "bada": nc.dram_tensor("bada", [2 * C], FP, kind="ExternalInput"),
        "wlin": nc.dram_tensor("wlin", [C, OUTD], FP, kind="ExternalInput"),
        "blin": nc.dram_tensor("blin", [OUTD], FP, kind="ExternalInput"),
        "yT": nc.dram_tensor("yT", [OUTD, STOK], FP, kind="ExternalOutput"),
    }


def build_nc():
    nc = bacc.Bacc("TRN2", target_bir_lowering=False, debug=False)
    io = declare_io(nc)
    with tile.TileContext(nc) as tc:
        _body(nc, tc, io)
    nc.compile()
    return nc


_CACHE = {}


def _get_nc():
    if "nc" not in _CACHE:
        _CACHE["nc"] = build_nc()
    return _CACHE["nc"]


def make_in_maps(x, v, t, c, wq, bq, wkv, bkv, wproj, bproj, wada, bada, wlin, blin):
    f32 = lambda a: np.ascontiguousarray(np.asarray(a, dtype=np.float32))
    x, v, t, c = f32(x), f32(v), f32(t), f32(c)
    wq, wkv, wproj, wada = f32(wq), f32(wkv), f32(wproj), f32(wada)

    def blocked(w):  # [cin, cout] -> [co_tile, ci_tile, 128, 128]
        co = w.shape[1] // P
        return np.ascontiguousarray(w.reshape(G, P, co, P).transpose(2, 0, 1, 3))

    shared = {
        "wq_b": blocked(wq),
        "bq": f32(bq),
        "wkvk_b": blocked(np.ascontiguousarray(wkv[:, :C])),
        "wkvv": f32(wkv[:, C:]),
        "bkv": f32(bkv),
        "wproj_b": blocked(wproj),
        "bproj": f32(bproj),
        "wada_b": blocked(wada),
        "bada": f32(bada),
        "wlin": f32(wlin),
        "blin": f32(blin),
    }
    in_maps = []
    for m in range(8):
        b, half = divmod(m, 2)
        in_maps.append(
            {
                "x_sl": f32(x[b, half * STOK : (half + 1) * STOK, :]),
                "v_b": f32(v[b]),
                "t_b": f32(t[b]),
                "c_sl": f32(c[b, half * F : (half + 1) * F, :]),
                **shared,
            }
        )
    return in_maps


def assemble_y(results):
    y = np.empty((B, T, NT, OUTD), np.float32)
    for m in range(8):
        b, half = divmod(m, 2)
        yt = np.asarray(results[m]["yT"])  # [OUTD, STOK]
        y[b, half * F : (half + 1) * F] = yt.T.reshape(F, NT, OUTD)
    return y


def kernel(x, v, t, c, wq, bq, wkv, bkv, wproj, bproj, wada, bada, wlin, blin, T=16, H=16):
    nc = _get_nc()
    in_maps = make_in_maps(
        x, v, t, c, wq, bq, wkv, bkv, wproj, bproj, wada, bada, wlin, blin
    )
    res = run_bass_kernel_spmd(nc, in_maps, core_ids=list(range(8)))
    return assemble_y(res.results)



# revision 2
# speedup vs baseline: 1.0806x; 1.0806x over previous
"""Trainium2 Bass kernel: cross-attention + adaLN-LN + linear block, fp8/bf16.

Sharding: 8 cores = 4 batches x 2 token-halves of S=4096 (as baseline).

Key differences from the fp32r baseline:
- All large GEMMs run as fp8e4m3 DoubleRow matmuls: two K=128 blocks packed
  side-by-side in the free dim of both operands, halving PE time per MAC.
  Contractions over C pair adjacent cin groups; attention scores use a
  zeroed second lhsT block (K=64 real); attn-out pairs adjacent key blocks.
- Weights are host-scaled by 32 so fp8 values sit in e4m3's normal range;
  the exp activation scale absorbs 32*32, the proj eviction absorbs 2^-11,
  and the softmax ones-column is 1/64 so reciprocals (64/denom ~ 0.09) stay
  normal in fp8.
- x/x1 master is bf16; LN sums, adaLN and the final linear run in bf16.
- Softmax: scores for a kb-pair land in a 2-bank PSUM tile and one Exp
  activation (free size 1024) converts them straight to fp8.
- Eviction work is split between the DVE and GpSimd engines; proj and LN
  chunks of the previous token block are interleaved between attention
  heads so the PE fills the gaps while the ACT engine streams exps.
- LN sqrt is deferred to one batched activation to avoid ACT table thrash.
"""

import sys

for _p in ("/opt/trn_rl_repo", "/opt/pypackages"):
    if _p not in sys.path:
        sys.path.append(_p)

import numpy as np
import ml_dtypes

import concourse.bacc as bacc
import concourse.tile as tile
from concourse import mybir
from concourse.bass_utils import run_bass_kernel_spmd
from concourse.masks import make_identity

FP = mybir.dt.float32
FPR = mybir.dt.float32r
BF = mybir.dt.bfloat16
F8 = mybir.dt.float8e4
AF = mybir.ActivationFunctionType
OP = mybir.AluOpType
DRM = mybir.MatmulPerfMode.DoubleRow

NP_BF = ml_dtypes.bfloat16
NP_F8 = ml_dtypes.float8_e4m3


def _r(ap):
    return ap.bitcast(FPR)


# Problem sizes (hardcoded per spec).
B = 4
S = 4096
C = 1024
N2 = 512
H = 16
D = 64
T = 16
NT = 256          # tokens per frame
OUTD = 32

STOK = S // 2     # tokens per core
F = 8             # frames per core
G = C // 128      # 8 channel groups
TB = 512          # token block
NTB = STOK // TB  # 4
KB = N2 // 128    # 4 key blocks
P = 128
SEG = 128         # vv cols per head-pair segment

WS = 32.0                      # host weight scale
EXP_SCALE = (D ** -0.5) / (WS * WS)
ONES_COL = 1.0 / 64.0          # softmax denom ride scale
PROJ_SCALE = 1.0 / (64.0 * WS)  # ao8 = 64*ao, wproj8 = 32*wproj
EPS = 1e-6


def _body(nc, tc, io):
    with nc.allow_low_precision("fp8/bf16 matmul operands"):
        _body_inner(nc, tc, io)


def _body_inner(nc, tc, io):
    xTbf, xT8d, vT8d = io["xTbf"], io["xT8"], io["vT8d"]
    tvec, cmat = io["t_b"], io["c_sl"]
    wq8, bq32 = io["wq8"], io["bq32"]
    wkvk8, wkvv8, bkvk32, bkvv8 = io["wkvk8"], io["wkvv8"], io["bkvk32"], io["bkvv8"]
    wproj8 = io["wproj8"]
    wada, bada = io["wada_bf"], io["bada"]
    wlin, blin = io["wlin_bf"], io["blin"]
    yT = io["yT"]

    def ev():
        return nc.vector  # all PSUM-reading evictions must be on DVE

    with (
        tc.tile_pool(name="consts", bufs=1) as consts,
        tc.tile_pool(name="xT", bufs=1) as xTp,
        tc.tile_pool(name="x8", bufs=1) as x8p,
        tc.tile_pool(name="q8", bufs=1) as q8p,
        tc.tile_pool(name="k8", bufs=1) as k8p,
        tc.tile_pool(name="vv", bufs=1) as vvp,
        tc.tile_pool(name="wp", bufs=4) as wp,
    ):
        # ---- constants ----
        scratch = consts.tile([P, P], FP, tag="scratch")
        make_identity(nc, scratch)
        ident = consts.tile([P, P], FP, tag="ident")
        nc.vector.tensor_copy(out=_r(ident), in_=scratch)
        ones8 = consts.tile([P, 2, P], F8, tag="ones8")
        nc.vector.memset(ones8[:, 0, :], 1.0)
        nc.vector.memset(ones8[:, 1, :], 0.0)
        ones_bf = consts.tile([P, 2], BF, tag="onesbf")
        nc.vector.memset(ones_bf, 1.0)
        ones_f = consts.tile([1, OUTD], FP, tag="onesf")
        nc.vector.memset(ones_f, 1.0)
        eps_t = consts.tile([1, 1], FP, tag="eps")
        nc.vector.memset(eps_t, EPS)

        bq_t = consts.tile([P, G], FP, tag="bq")
        nc.sync.dma_start(out=bq_t, in_=bq32.ap().rearrange("(g p) -> p g", p=P))
        bkvk_t = consts.tile([P, G], FP, tag="bkvk")
        nc.sync.dma_start(out=bkvk_t, in_=bkvk32.ap().rearrange("(g p) -> p g", p=P))
        bkvv_t = consts.tile([1, 2, 2, TB], F8, tag="bkvv")
        nc.sync.dma_start(out=bkvv_t, in_=bkvv8.ap())
        bada_t = consts.tile([P, 16], FP, tag="bada")
        nc.sync.dma_start(out=bada_t, in_=bada.ap().rearrange("(g p) -> p g", p=P))
        blin_row = consts.tile([1, OUTD], FP, tag="blin")
        nc.sync.dma_start(
            out=blin_row, in_=blin.ap().rearrange("(one o) -> one o", one=1)
        )
        t_t = consts.tile([P, G], FP, tag="tvec")
        nc.sync.dma_start(out=t_t, in_=tvec.ap().rearrange("(g p) -> p g", p=P))
        wlin_sb = consts.tile([P, G, OUTD], BF, tag="wlin")
        nc.sync.dma_start(
            out=wlin_sb, in_=wlin.ap().rearrange("(ci p) o -> p ci o", p=P)
        )
        watall = consts.tile([P, 16, G, P], BF, tag="watall")
        silu_t = consts.tile([P, G, F], BF, tag="silu")
        ada_t = consts.tile([P, 16, F], FP, tag="ada")
        ada_bf = consts.tile([P, 16, F], BF, tag="adabf")
        selc_t = consts.tile([P, 16, 2, 16], F8, tag="selc")
        nc.sync.dma_start(out=selc_t, in_=io["selc8"].ap())
        selp_t = consts.tile([16, G, 2, P], F8, tag="selp")
        nc.sync.dma_start(out=selp_t, in_=io["selp8"].ap())
        dnb16 = consts.tile([16, 2, TB], F8, tag="dnb16")
        nc.vector.memset(dnb16, 0.0)
        mu_all = consts.tile([1, NTB, TB], FP, tag="mu")
        var_all = consts.tile([1, NTB, TB], FP, tag="var")
        std_all = consts.tile([1, NTB, TB], FP, tag="std")
        rst_all = consts.tile([1, NTB, TB], FP, tag="rst")

        # ---- persistent activations (hosts supplies transposed x) ----
        xT_bf = xTp.tile([P, G, STOK], BF, tag="xT")
        x8_t = x8p.tile([P, G, STOK], F8, tag="x8")
        q8_t = q8p.tile([P, G, STOK + TB], F8, tag="q8")   # +pad for rhs blocks
        k8_t = k8p.tile([P, G, KB, 2, P], F8, tag="k8")
        vv8 = vvp.tile([P, KB, G, SEG], F8, tag="vv8")

        # zero-fill the regions matmuls read but evictions never write
        for gi in range(4):
            gsl = slice(2 * gi, 2 * gi + 2)
            nc.gpsimd.dma_start(
                out=x8_t[:, gsl, :],
                in_=xT8d.ap().rearrange("(g p) t -> p g t", p=P)[:, gsl, :],
            )
        nc.vector.memset(q8_t[:, :, STOK:], 0.0)           # rhs pad blocks
        nc.vector.memset(k8_t[:, :, :, 1, :], 0.0)         # lhsT zero blocks

        with (
            tc.tile_pool(name="psA", bufs=4, space="PSUM") as psA,
            tc.tile_pool(name="psQ", bufs=2, space="PSUM") as psQ,
        ):
            with tc.tile_pool(name="vT", bufs=1) as vTp:
                vT8 = vTp.tile([P, G, N2], F8, tag="vT8")

                nc.sync.dma_start(
                    out=vT8, in_=vT8d.ap().rearrange("(g p) n -> p g n", p=P)
                )
                with tc.tile_pool(name="ld", bufs=2) as ldp:
                    # ---- vv build: DR over cin pairs, bias ridden, /32 evict ----
                    for half in range(2):
                        wvt = wp.tile([P, 4, 2, TB], F8, name="wv", tag="w")
                        nc.sync.dma_start(out=wvt, in_=wkvv8.ap()[half])
                        pss = [
                            psA.tile([P, TB], FP, name="psv", tag="ps")
                            for _ in range(KB)
                        ]
                        for j in range(4):
                            for kb in range(KB):
                                nc.tensor.matmul(
                                    pss[kb],
                                    lhsT=vT8[:, 2 * j : 2 * j + 2, kb * P : (kb + 1) * P],
                                    rhs=wvt[:, j],
                                    start=(j == 0),
                                    stop=False,
                                    perf_mode=DRM,
                                )
                        for kb in range(KB):
                            nc.tensor.matmul(
                                pss[kb],
                                lhsT=ones8[0:1, :, :],
                                rhs=bkvv_t[0:1, half],
                                start=False,
                                stop=True,
                                perf_mode=DRM,
                            )
                            gs = slice(half * 4, half * 4 + 4)
                            ev().tensor_scalar_mul(
                                vv8[:, kb, gs, :],
                                pss[kb].rearrange("p (a j) -> p a j", j=SEG),
                                1.0 / WS,
                            )

                    # ---- kT: DR over cin pairs -> k8 (+bias, fp8) ----
                    for g in range(G):
                        wkt = wp.tile([P, 4, 2, P], F8, name="wk", tag="w")
                        nc.sync.dma_start(out=wkt, in_=wkvk8.ap()[g])
                        psk = psA.tile([P, N2], FP, name="psk", tag="ps")
                        for j in range(4):
                            nc.tensor.matmul(
                                psk,
                                lhsT=wkt[:, j],
                                rhs=vT8[:, 2 * j : 2 * j + 2, :],
                                start=(j == 0),
                                stop=(j == 3),
                                perf_mode=DRM,
                            )
                        ev().tensor_scalar_add(
                            k8_t[:, g, :, 0, :],
                            psk.rearrange("p (kb c) -> p kb c", c=P),
                            bkvk_t[:, g : g + 1],
                        )

                with tc.tile_pool(name="ldx", bufs=2) as ldx:
                    # ---- adaLN: silu(t + c) @ wada + bada (bf16) ----
                    c_nat = ldx.tile([F, C], FP, name="cnat", tag="misc", bufs=1)
                    nc.sync.dma_start(out=_r(c_nat), in_=_r(cmat[:, :]))
                    for g in range(G):
                        pt = psA.tile([P, F], FP, name="ptc", tag="ps")
                        nc.tensor.transpose(
                            _r(pt), _r(c_nat[:, g * P : (g + 1) * P]), _r(ident[0:F, 0:F])
                        )
                        nc.scalar.activation(
                            out=silu_t[:, g, :],
                            in_=pt,
                            func=AF.Silu,
                            bias=t_t[:, g : g + 1],
                            scale=1.0,
                        )

                # ---- q projection: DR over cin pairs -> q8 (+bias) ----
                for g in range(G):
                    wqt = wp.tile([P, 4, 2, P], F8, name="wq", tag="w")
                    nc.sync.dma_start(out=wqt, in_=wq8.ap()[g])
                    pst2 = [
                        psQ.tile([P, 2, TB], FP, name=f"psq{i}", tag="psq")
                        for i in range(2)
                    ]
                    for j in range(4):
                        for tb in range(NTB):
                            nc.tensor.matmul(
                                pst2[tb // 2][:, tb % 2, :],
                                lhsT=wqt[:, j],
                                rhs=x8_t[:, 2 * j : 2 * j + 2, tb * TB : (tb + 1) * TB],
                                start=(j == 0),
                                stop=(j == 3),
                                perf_mode=DRM,
                            )
                    for i in range(2):
                        if (2 * g + i) % 2 == 0:
                            nc.scalar.activation(
                                out=q8_t[:, g, i * 2 * TB : (i + 1) * 2 * TB].rearrange(
                                    "p (a n) -> p a n", n=TB
                                ),
                                in_=pst2[i],
                                func=AF.Identity,
                                bias=bq_t[:, g : g + 1],
                                scale=1.0,
                            )
                        else:
                            nc.vector.tensor_scalar_add(
                                q8_t[:, g, i * 2 * TB : (i + 1) * 2 * TB].rearrange(
                                    "p (a n) -> p a n", n=TB
                                ),
                                pst2[i],
                                bq_t[:, g : g + 1],
                            )

        # ---- attention, with prev-tb normalize/proj/LN interleaved ----
        with (
            tc.tile_pool(name="sh", bufs=2, space="PSUM") as shp,
            tc.tile_pool(name="ex", bufs=2) as exp_pool,
            tc.tile_pool(name="sq", bufs=2) as sqp,
            tc.tile_pool(name="qbf", bufs=1) as qbfp,
        ):
            qbf = qbfp.tile([P, G, TB], BF, tag="qbf")
            sh_tiles = {}

            def norm_chunk(tb, g, pool):
                """q8[:, g, tb] = qbf (raw attn out, bf16) * 64/denom."""
                tbs = slice(tb * TB, (tb + 1) * TB)
                bc_ps = pool.tile([P, TB], FP, name="bcp", tag="pj")
                nc.tensor.matmul(
                    bc_ps,
                    lhsT=selp_t[:, g, :, :],
                    rhs=dnb16,
                    start=True,
                    stop=True,
                    perf_mode=DRM,
                )
                nc.vector.tensor_mul(q8_t[:, g, tbs], qbf[:, g, :], bc_ps)

            def proj_chunk(tb, g, pool):
                tbs = slice(tb * TB, (tb + 1) * TB)
                wpt = wp.tile([P, 4, 2, P], F8, name="wp8", tag="w")
                nc.sync.dma_start(out=wpt, in_=wproj8.ap()[g])
                pst = pool.tile([P, TB], FP, name="psp", tag="pj")
                for j in range(4):
                    nc.tensor.matmul(
                        pst,
                        lhsT=wpt[:, j],
                        rhs=q8_t[:, 2 * j : 2 * j + 2, tbs],
                        start=(j == 0),
                        stop=(j == 3),
                        perf_mode=DRM,
                    )
                nc.vector.scalar_tensor_tensor(
                    out=xT_bf[:, g, tbs],
                    in0=pst,
                    scalar=PROJ_SCALE,
                    in1=xT_bf[:, g, tbs],
                    op0=OP.mult,
                    op1=OP.add,
                )

            def ada_chunk(ct, pool):
                pa = pool.tile([P, TB], FP, name="pta", tag="pj")
                for ci in range(G):
                    nc.tensor.matmul(
                        pa[:, 0:F],
                        lhsT=watall[:, ct, ci, :],
                        rhs=silu_t[:, ci, :],
                        start=(ci == 0),
                        stop=(ci == G - 1),
                    )
                if ct < 8:
                    nc.vector.tensor_scalar_add(
                        ada_t[:, ct, :], pa[:, 0:F], bada_t[:, ct : ct + 1]
                    )
                else:
                    nc.vector.tensor_scalar(
                        ada_t[:, ct, :],
                        pa[:, 0:F],
                        bada_t[:, ct : ct + 1],
                        1.0,
                        op0=OP.add,
                        op1=OP.add,
                    )

            def ln_chunk(tb, g, sh, act_sq=False):
                """Accumulate sum(x1) into sh row 32, sum(x1^2) into row 64."""
                tbs = slice(tb * TB, (tb + 1) * TB)
                sqt = sqp.tile([P, TB], BF, tag="sq")
                if act_sq:
                    nc.scalar.activation(
                        out=sqt, in_=xT_bf[:, g, tbs], func=AF.Square, scale=1.0
                    )
                else:
                    nc.vector.tensor_mul(sqt, xT_bf[:, g, tbs], xT_bf[:, g, tbs])
                nc.tensor.matmul(
                    sh[32:33, :],
                    lhsT=ones_bf[:, 0:1],
                    rhs=xT_bf[:, g, tbs],
                    start=(g == 0),
                    stop=(g == G - 1),
                )
                nc.tensor.matmul(
                    sh[64:65, :],
                    lhsT=ones_bf[:, 0:1],
                    rhs=sqt,
                    start=(g == 0),
                    stop=(g == G - 1),
                )
                if g == G - 1:
                    mu = mu_all[0:1, tb, :]
                    nc.vector.tensor_scalar_mul(_r(mu), sh[32:33, :], 1.0 / C)
                    musq = sqp.tile([1, TB], FP, name="musq", tag="mu2")
                    nc.vector.tensor_mul(musq, mu, mu)
                    nc.vector.scalar_tensor_tensor(
                        out=var_all[0:1, tb, :],
                        in0=sh[64:65, :],
                        scalar=1.0 / C,
                        in1=musq,
                        op0=OP.mult,
                        op1=OP.subtract,
                    )

            with (
                tc.tile_pool(name="sc", bufs=2, space="PSUM") as scp,
                tc.tile_pool(name="ao", bufs=1, space="PSUM") as aop,
                tc.tile_pool(name="pj", bufs=1, space="PSUM") as pjp,
            ):
                for tb in range(NTB):
                    if tb == 0:
                        for ci4 in range(4):
                            nc.sync.dma_start(
                                out=watall[:, 4 * ci4 : 4 * ci4 + 4, :, :],
                                in_=wada.ap()[4 * ci4 : 4 * ci4 + 4].rearrange(
                                    "c p g k -> p c g k"
                                ),
                            )
                        for gi in range(4):
                            gsl = slice(2 * gi, 2 * gi + 2)
                            nc.sync.dma_start(
                                out=xT_bf[:, gsl, :],
                                in_=xTbf.ap().rearrange(
                                    "(g p) t -> p g t", p=P
                                )[:, gsl, :],
                            )
                    sh = shp.tile([P, TB], FP, name="shb", tag="sh")
                    sh_tiles[tb] = sh
                    for g in range(G):
                        ao_pair = aop.tile([P, TB], FP, name="aop", tag="ao")
                        # odd half first: its DR lhsT must span the full 128
                        # out partitions (DR rejects column tile offsets);
                        # the even half then re-zeroes rows 0:63.
                        for si, half in enumerate((1, 0)):
                            h2 = 2 * g + half
                            r0 = half * 64
                            ex8 = exp_pool.tile([P, KB, TB], F8, tag="ex")
                            rhs_q = q8_t[
                                r0 : r0 + 64, g, tb * TB : (tb + 2) * TB
                            ].rearrange("p (two n) -> p two n", two=2)
                            for jp in range(2):
                                sc = scp.tile([P, 2, TB], FP, name="scs", tag="sc")
                                for kk in range(2):
                                    kb = 2 * jp + kk
                                    nc.tensor.matmul(
                                        sc[:, kk, :],
                                        lhsT=k8_t[r0 : r0 + 64, g, kb, :, :],
                                        rhs=rhs_q,
                                        start=True,
                                        stop=True,
                                        perf_mode=DRM,
                                    )
                                nc.scalar.activation(
                                    out=ex8[:, 2 * jp : 2 * jp + 2, :],
                                    in_=sc,
                                    func=AF.Exp,
                                    scale=EXP_SCALE,
                                )
                            if half == 1:
                                ao_out, ao_lo, ao_hi = ao_pair, 0, P
                            else:
                                ao_out, ao_lo, ao_hi = ao_pair[0:64, :], 0, 64
                            for jp in range(2):
                                nc.tensor.matmul(
                                    ao_out,
                                    lhsT=vv8[:, 2 * jp : 2 * jp + 2, g, ao_lo:ao_hi],
                                    rhs=ex8[:, 2 * jp : 2 * jp + 2, :],
                                    start=(jp == 0),
                                    stop=(jp == 1),
                                    perf_mode=DRM,
                                )
                                nc.tensor.matmul(
                                    sh[0:16, :],
                                    lhsT=selc_t[:, h2, :, :],
                                    rhs=ex8[:, 2 * jp : 2 * jp + 2, :],
                                    start=(g == 0 and si == 0 and jp == 0),
                                    stop=(g == G - 1 and si == 1 and jp == 1),
                                    perf_mode=DRM,
                                )
                            # prev-tb chunks ride the ACT-bound head slots
                            slot = 2 * g + si
                            if tb > 0:
                                if slot < 8:
                                    norm_chunk(tb - 1, slot, pjp)
                                else:
                                    proj_chunk(tb - 1, slot - 8, pjp)
                                    ln_chunk(tb - 1, slot - 8, sh)
                            elif slot >= 8:
                                ada_chunk(2 * (slot - 8), pjp)
                                ada_chunk(2 * (slot - 8) + 1, pjp)
                        # stage raw attn-out pair (denominator not yet known)
                        nc.vector.tensor_copy(out=qbf[:, g, :], in_=ao_pair)
                    # all 32 collector matmuls done: one reciprocal per tb
                    nc.vector.reciprocal(out=dnb16[:, 0, :], in_=sh[0:16, :])

            # tail: overlap last-tb normalize/proj/LN with the f-loop for
            # the first three token blocks (their LN stats are final); the
            # batched sqrt is split 3+1 so only f6/f7 wait on the last tb.
            with (
                tc.tile_pool(name="psT", bufs=3, space="PSUM") as psT,
                tc.tile_pool(name="psD", bufs=3, space="PSUM") as psD,
                tc.tile_pool(name="st", bufs=2) as stp,
                tc.tile_pool(name="w1", bufs=2) as w1p,
                tc.tile_pool(name="rows", bufs=4) as rowp,
                tc.tile_pool(name="yo", bufs=2) as yop,
            ):
                def stats_block(tb_lo, tb_hi):
                    n = tb_hi - tb_lo
                    nc.scalar.activation(
                        out=_r(std_all[0:1, tb_lo:tb_hi, :].rearrange(
                            "one a n -> one (a n)"
                        )),
                        in_=var_all[0:1, tb_lo:tb_hi, :].rearrange(
                            "one a n -> one (a n)"
                        ),
                        func=AF.Sqrt,
                        bias=eps_t[0:1, :],
                        scale=1.0,
                    )
                    nc.vector.reciprocal(
                        out=_r(rst_all[0:1, tb_lo:tb_hi, :].rearrange(
                            "one a n -> one (a n)"
                        )),
                        in_=std_all[0:1, tb_lo:tb_hi, :].rearrange(
                            "one a n -> one (a n)"
                        ),
                    )

                def f_block(tb):
                    for f2 in range(2):
                        f = tb * 2 + f2
                        fcs = slice(f2 * NT, (f2 + 1) * NT)
                        gcs = slice(tb * TB + f2 * NT, tb * TB + (f2 + 1) * NT)
                        w1 = w1p.tile([P, G, OUTD], BF, tag="w1")
                        for g in range(G):
                            nc.vector.tensor_scalar_mul(
                                w1[:, g, :],
                                wlin_sb[:, g, :],
                                ada_t[:, 8 + g, f : f + 1],
                            )
                        ws1_ps = psD.tile([1, OUTD], FP, name="ws1", tag="ps")
                        c2_ps = psD.tile([1, OUTD], FP, name="c2", tag="ps")
                        for g in range(G):
                            nc.tensor.matmul(
                                ws1_ps,
                                lhsT=ada_bf[:, 8 + g, f : f + 1],
                                rhs=wlin_sb[:, g, :],
                                start=(g == 0),
                                stop=(g == G - 1),
                            )
                            nc.tensor.matmul(
                                c2_ps,
                                lhsT=ada_bf[:, g, f : f + 1],
                                rhs=wlin_sb[:, g, :],
                                start=(g == 0),
                                stop=(g == G - 1),
                            )
                        ws1n = rowp.tile([1, OUTD], FP, name="ws1n", tag="rows")
                        c2b = rowp.tile([1, OUTD], FP, name="c2b", tag="rows")
                        nc.vector.tensor_scalar_mul(_r(ws1n), ws1_ps, -1.0)
                        nc.vector.tensor_tensor(_r(c2b), c2_ps, blin_row, OP.add)
                        bc32_ps = psD.tile([OUTD, NT], FP, name="bc32", tag="ps")
                        nc.tensor.matmul(
                            bc32_ps,
                            lhsT=_r(ones_f),
                            rhs=_r(rst_all[0:1, tb, fcs]),
                            start=True,
                            stop=True,
                        )
                        bc32_sb = yop.tile([OUTD, NT], FP, name="bc32", tag="bc32")
                        nc.vector.tensor_copy(out=bc32_sb, in_=bc32_ps)
                        y_ps = psD.tile([OUTD, NT], FP, name="yps", tag="ps")
                        for g in range(G):
                            nc.tensor.matmul(
                                y_ps,
                                lhsT=w1[:, g, :],
                                rhs=xT_bf[:, g, gcs],
                                start=(g == 0),
                                stop=False,
                            )
                        nc.tensor.matmul(
                            y_ps,
                            lhsT=_r(ws1n),
                            rhs=_r(mu_all[0:1, tb, fcs]),
                            start=False,
                            stop=False,
                        )
                        nc.tensor.matmul(
                            y_ps,
                            lhsT=_r(c2b),
                            rhs=_r(std_all[0:1, tb, fcs]),
                            start=False,
                            stop=True,
                        )
                        yt = yop.tile([OUTD, NT], FP, tag="y")
                        nc.vector.tensor_mul(yt, y_ps, bc32_sb)
                        nc.sync.dma_start(out=yT[:, gcs], in_=yt)

                nc.vector.tensor_copy(out=ada_bf, in_=ada_t)
                stats_block(0, NTB - 1)
                for g in range(G):
                    norm_chunk(NTB - 1, g, psT)
                for g in range(G):
                    proj_chunk(NTB - 1, g, psT)
                for g in range(G):
                    ln_chunk(NTB - 1, g, sh_tiles[NTB - 1], act_sq=True)
                for tb in range(NTB - 1):
                    f_block(tb)
                stats_block(NTB - 1, NTB)
                f_block(NTB - 1)


def declare_io(nc):
    return {
        "xTbf": nc.dram_tensor("xTbf", [C, STOK], BF, kind="ExternalInput"),
        "xT8": nc.dram_tensor("xT8", [C, STOK], F8, kind="ExternalInput"),
        "vT8d": nc.dram_tensor("vT8d", [C, N2], F8, kind="ExternalInput"),
        "t_b": nc.dram_tensor("t_b", [C], FP, kind="ExternalInput"),
        "c_sl": nc.dram_tensor("c_sl", [F, C], FP, kind="ExternalInput"),
        "wq8": nc.dram_tensor("wq8", [G, P, 4, 2, P], F8, kind="ExternalInput"),
        "bq32": nc.dram_tensor("bq32", [C], FP, kind="ExternalInput"),
        "wkvk8": nc.dram_tensor("wkvk8", [G, P, 4, 2, P], F8, kind="ExternalInput"),
        "wkvv8": nc.dram_tensor("wkvv8", [2, P, 4, 2, TB], F8, kind="ExternalInput"),
        "bkvk32": nc.dram_tensor("bkvk32", [C], FP, kind="ExternalInput"),
        "bkvv8": nc.dram_tensor("bkvv8", [1, 2, 2, TB], F8, kind="ExternalInput"),
        "wproj8": nc.dram_tensor("wproj8", [G, P, 4, 2, P], F8, kind="ExternalInput"),
        "selc8": nc.dram_tensor("selc8", [P, 16, 2, 16], F8, kind="ExternalInput"),
        "selp8": nc.dram_tensor("selp8", [16, G, 2, P], F8, kind="ExternalInput"),
        "wada_bf": nc.dram_tensor("wada_bf", [16, P, G, P], BF, kind="ExternalInput"),
        "bada": nc.dram_tensor("bada", [2 * C], FP, kind="ExternalInput"),
        "wlin_bf": nc.dram_tensor("wlin_bf", [C, OUTD], BF, kind="ExternalInput"),
        "blin": nc.dram_tensor("blin", [OUTD], FP, kind="ExternalInput"),
        "yT": nc.dram_tensor("yT", [OUTD, STOK], FP, kind="ExternalOutput"),
    }


def build_nc():
    nc = bacc.Bacc("TRN2", target_bir_lowering=False, debug=False)
    io = declare_io(nc)
    with tile.TileContext(nc) as tc:
        _body(nc, tc, io)
    nc.compile()
    return nc


_CACHE = {}


def _get_nc():
    if "nc" not in _CACHE:
        _CACHE["nc"] = build_nc()
    return _CACHE["nc"]


def _block_w(w):
    """[cin=1024, cout] -> [cout_grp, p, j, b, cout_col] for DR lhsT tiles."""
    co = w.shape[1] // P
    return np.ascontiguousarray(
        w.reshape(4, 2, P, co, P).transpose(3, 2, 0, 1, 4)
    )


def make_in_maps(x, v, t, c, wq, bq, wkv, bkv, wproj, bproj, wada, bada, wlin, blin):
    f32 = lambda a: np.ascontiguousarray(np.asarray(a, dtype=np.float32))
    x, v, t, c = f32(x), f32(v), f32(t), f32(c)
    wq, wkv, wproj, wada = f32(wq), f32(wkv), f32(wproj), f32(wada)
    bq, bkv, bproj, bada = f32(bq), f32(bkv), f32(bproj), f32(bada)
    wlin, blin = f32(wlin), f32(blin)

    wkvv = np.ascontiguousarray(wkv[:, C:])
    # [half, p, j, b, n]
    wkvv8 = np.ascontiguousarray(
        (WS * wkvv).reshape(4, 2, P, 2, TB).transpose(3, 2, 0, 1, 4)
    ).astype(NP_F8)
    bkvv8 = np.zeros((1, 2, 2, TB), np.float32)
    bkvv8[0, :, 0, :] = (WS * bkv[C:]).reshape(2, TB)
    wada_b = np.ascontiguousarray(
        wada.reshape(G, P, 16, P).transpose(2, 1, 0, 3)
    ).astype(NP_BF)
    selc = np.zeros((P, 16, 2, 16), np.float32)
    for h in range(16):
        selc[:, h, :, h] = ONES_COL
    selp = np.zeros((16, G, 2, P), np.float32)
    for g in range(G):
        selp[2 * g, g, 0, 0:64] = 1.0
        selp[2 * g + 1, g, 0, 64:128] = 1.0

    shared = {
        "wq8": _block_w(WS * wq).astype(NP_F8),
        "bq32": f32(WS * bq),
        "wkvk8": _block_w(WS * wkv[:, :C]).astype(NP_F8),
        "wkvv8": wkvv8,
        "bkvk32": f32(WS * bkv[:C]),
        "bkvv8": bkvv8.astype(NP_F8),
        "wproj8": _block_w(WS * wproj).astype(NP_F8),
        "selc8": selc.astype(NP_F8),
        "selp8": selp.astype(NP_F8),
        "wada_bf": wada_b,
        "bada": f32(bada),
        "wlin_bf": wlin.astype(NP_BF),
        "blin": f32(blin),
    }
    in_maps = []
    vT8_cache = {}
    for m in range(8):
        b, half = divmod(m, 2)
        xs = x[b, half * STOK : (half + 1) * STOK, :]
        if b not in vT8_cache:
            vT8_cache[b] = np.ascontiguousarray(v[b].T).astype(NP_F8)
        in_maps.append(
            {
                "xTbf": np.ascontiguousarray((xs + bproj[None, :]).T).astype(NP_BF),
                "xT8": np.ascontiguousarray(xs.T).astype(NP_F8),
                "vT8d": vT8_cache[b],
                "t_b": f32(t[b]),
                "c_sl": f32(c[b, half * F : (half + 1) * F, :]),
                **shared,
            }
        )
    return in_maps


def assemble_y(results):
    y = np.empty((B, T, NT, OUTD), np.float32)
    for m in range(8):
        b, half = divmod(m, 2)
        yt = np.asarray(results[m]["yT"])  # [OUTD, STOK]
        y[b, half * F : (half + 1) * F] = yt.T.reshape(F, NT, OUTD)
    return y


def kernel(x, v, t, c, wq, bq, wkv, bkv, wproj, bproj, wada, bada, wlin, blin, T=16, H=16):
    nc = _get_nc()
    in_maps = make_in_maps(
        x, v, t, c, wq, bq, wkv, bkv, wproj, bproj, wada, bada, wlin, blin
    )
    res = run_bass_kernel_spmd(nc, in_maps, core_ids=list(range(8)))
    return assemble_y(res.results)


# revision 3
# speedup vs baseline: 4.6339x; 4.2882x over previous
"""Trainium2 Bass kernel: cross-attention + adaLN-LN + linear block, fp8/bf16.

Sharding: 8 cores = 4 batches x 2 token-halves of S=4096 (as baseline).

Key differences from the fp32r baseline:
- All large GEMMs run as fp8e4m3 DoubleRow matmuls: two K=128 blocks packed
  side-by-side in the free dim of both operands, halving PE time per MAC.
  Contractions over C pair adjacent cin groups; attention scores use a
  zeroed second lhsT block (K=64 real); attn-out pairs adjacent key blocks.
- Weights are host-scaled by 32 so fp8 values sit in e4m3's normal range;
  the exp activation scale absorbs 32*32, the proj eviction absorbs 2^-11,
  and the softmax ones-column is 1/64 so reciprocals (64/denom ~ 0.09) stay
  normal in fp8.
- x/x1 master is bf16; LN sums, adaLN and the final linear run in bf16.
- Softmax: scores for a kb-pair land in a 2-bank PSUM tile and one Exp
  activation (free size 1024) converts them straight to fp8.
- Eviction work is split between the DVE and GpSimd engines; proj and LN
  chunks of the previous token block are interleaved between attention
  heads so the PE fills the gaps while the ACT engine streams exps.
- LN sqrt is deferred to one batched activation to avoid ACT table thrash.
"""

import sys

for _p in ("/opt/trn_rl_repo", "/opt/pypackages"):
    if _p not in sys.path:
        sys.path.append(_p)

import numpy as np
import ml_dtypes

import concourse.bacc as bacc
import concourse.tile as tile
from concourse import mybir
from concourse.bass_utils import run_bass_kernel_spmd
from concourse.masks import make_identity

FP = mybir.dt.float32
FPR = mybir.dt.float32r
BF = mybir.dt.bfloat16
F8 = mybir.dt.float8e4
AF = mybir.ActivationFunctionType
OP = mybir.AluOpType
DRM = mybir.MatmulPerfMode.DoubleRow

NP_BF = ml_dtypes.bfloat16
NP_F8 = ml_dtypes.float8_e4m3


def _r(ap):
    return ap.bitcast(FPR)


# Problem sizes (hardcoded per spec).
B = 4
S = 4096
C = 1024
N2 = 512
H = 16
D = 64
T = 16
NT = 256          # tokens per frame
OUTD = 32

STOK = S // 2     # tokens per core
F = 8             # frames per core
G = C // 128      # 8 channel groups
TB = 512          # token block
NTB = STOK // TB  # 4
KB = N2 // 128    # 4 key blocks
P = 128
SEG = 128         # vv cols per head-pair segment

WS = 32.0                      # host weight scale
EXP_SCALE = (D ** -0.5) / (WS * WS)
ONES_COL = 1.0 / 64.0          # softmax denom ride scale
PROJ_SCALE = 1.0 / (64.0 * WS)  # ao8 = 64*ao, wproj8 = 32*wproj
EPS = 1e-6

# ---- packed input blobs: one dram tensor per dtype to minimize the
# per-input dispatch overhead (measured ~40us/input through this stack) ----
_F8_SECTS = [
    ("xT8", (C, STOK)),
    ("vT8d", (C, N2)),
    ("wq8", (G, P, 4, 2, P)),
    ("wkvk8", (G, P, 4, 2, P)),
    ("wkvv8", (2, P, 4, 2, TB)),
    ("wproj8", (G, P, 4, 2, P)),
    ("bkvv8", (1, 2, 2, TB)),
    ("selc8", (P, 16, 2, 16)),
    ("selp8", (16, G, 2, P)),
]
_BF_SECTS = [
    ("xTbf", (C, STOK)),
    ("wada_bf", (16, P, G, P)),
    ("wlin_bf", (C, OUTD)),
]
_F32_SECTS = [
    ("t_b", (C,)),
    ("c_sl", (F, C)),
    ("bq32", (C,)),
    ("bkvk32", (C,)),
    ("bada", (2 * C,)),
    ("blin", (OUTD,)),
]


def _offsets(sects):
    out, off = {}, 0
    for name, shape in sects:
        n = int(np.prod(shape))
        out[name] = (off, n, shape)
        off += n
    return out, off


_F8_OFF, _F8_TOT = _offsets(_F8_SECTS)
_BF_OFF, _BF_TOT = _offsets(_BF_SECTS)
_F32_OFF, _F32_TOT = _offsets(_F32_SECTS)


def _body(nc, tc, io):
    with nc.allow_low_precision("fp8/bf16 matmul operands"):
        _body_inner(nc, tc, io)


def _body_inner(nc, tc, io):
    io = _sections(io)
    xTbf, xT8d, vT8d = io["xTbf"], io["xT8"], io["vT8d"]
    tvec, cmat = io["t_b"], io["c_sl"]
    wq8, bq32 = io["wq8"], io["bq32"]
    wkvk8, wkvv8, bkvk32, bkvv8 = io["wkvk8"], io["wkvv8"], io["bkvk32"], io["bkvv8"]
    wproj8 = io["wproj8"]
    wada, bada = io["wada_bf"], io["bada"]
    wlin, blin = io["wlin_bf"], io["blin"]
    yT = io["yT"]

    def ev():
        return nc.vector  # all PSUM-reading evictions must be on DVE

    with (
        tc.tile_pool(name="consts", bufs=1) as consts,
        tc.tile_pool(name="xT", bufs=1) as xTp,
        tc.tile_pool(name="x8", bufs=1) as x8p,
        tc.tile_pool(name="q8", bufs=1) as q8p,
        tc.tile_pool(name="k8", bufs=1) as k8p,
        tc.tile_pool(name="vv", bufs=1) as vvp,
        tc.tile_pool(name="wp", bufs=4) as wp,
    ):
        # ---- constants ----
        scratch = consts.tile([P, P], FP, tag="scratch")
        make_identity(nc, scratch)
        ident = consts.tile([P, P], FP, tag="ident")
        nc.vector.tensor_copy(out=_r(ident), in_=scratch)
        ones8 = consts.tile([P, 2, P], F8, tag="ones8")
        nc.vector.memset(ones8[:, 0, :], 1.0)
        nc.vector.memset(ones8[:, 1, :], 0.0)
        ones_bf = consts.tile([P, 2], BF, tag="onesbf")
        nc.vector.memset(ones_bf, 1.0)
        ones_f = consts.tile([1, OUTD], FP, tag="onesf")
        nc.vector.memset(ones_f, 1.0)
        eps_t = consts.tile([1, 1], FP, tag="eps")
        nc.vector.memset(eps_t, EPS)

        bq_t = consts.tile([P, G], FP, tag="bq")
        nc.sync.dma_start(out=bq_t, in_=bq32.ap().rearrange("(g p) -> p g", p=P))
        bkvk_t = consts.tile([P, G], FP, tag="bkvk")
        nc.sync.dma_start(out=bkvk_t, in_=bkvk32.ap().rearrange("(g p) -> p g", p=P))
        bkvv_t = consts.tile([1, 2, 2, TB], F8, tag="bkvv")
        nc.sync.dma_start(out=bkvv_t, in_=bkvv8.ap())
        bada_t = consts.tile([P, 16], FP, tag="bada")
        nc.sync.dma_start(out=bada_t, in_=bada.ap().rearrange("(g p) -> p g", p=P))
        blin_row = consts.tile([1, OUTD], FP, tag="blin")
        nc.sync.dma_start(
            out=blin_row, in_=blin.ap().rearrange("(one o) -> one o", one=1)
        )
        t_t = consts.tile([P, G], FP, tag="tvec")
        nc.sync.dma_start(out=t_t, in_=tvec.ap().rearrange("(g p) -> p g", p=P))
        wlin_sb = consts.tile([P, G, OUTD], BF, tag="wlin")
        nc.sync.dma_start(
            out=wlin_sb, in_=wlin.ap().rearrange("(ci p) o -> p ci o", p=P)
        )
        watall = consts.tile([P, 16, G, P], BF, tag="watall")
        silu_t = consts.tile([P, G, F], BF, tag="silu")
        ada_t = consts.tile([P, 16, F], FP, tag="ada")
        ada_bf = consts.tile([P, 16, F], BF, tag="adabf")
        selc_t = consts.tile([P, 16, 2, 16], F8, tag="selc")
        nc.sync.dma_start(out=selc_t, in_=io["selc8"].ap())
        selp_t = consts.tile([16, G, 2, P], F8, tag="selp")
        nc.sync.dma_start(out=selp_t, in_=io["selp8"].ap())
        dnb16 = consts.tile([16, 2, TB], F8, tag="dnb16")
        nc.vector.memset(dnb16, 0.0)
        mu_all = consts.tile([1, NTB, TB], FP, tag="mu")
        var_all = consts.tile([1, NTB, TB], FP, tag="var")
        std_all = consts.tile([1, NTB, TB], FP, tag="std")
        rst_all = consts.tile([1, NTB, TB], FP, tag="rst")

        # ---- persistent activations (hosts supplies transposed x) ----
        xT_bf = xTp.tile([P, G, STOK], BF, tag="xT")
        x8_t = x8p.tile([P, G, STOK], F8, tag="x8")
        q8_t = q8p.tile([P, G, STOK + TB], F8, tag="q8")   # +pad for rhs blocks
        k8_t = k8p.tile([P, G, KB, 2, P], F8, tag="k8")
        vv8 = vvp.tile([P, KB, G, SEG], F8, tag="vv8")

        # zero-fill the regions matmuls read but evictions never write
        for gi in range(4):
            gsl = slice(2 * gi, 2 * gi + 2)
            nc.gpsimd.dma_start(
                out=x8_t[:, gsl, :],
                in_=xT8d.ap().rearrange("(g p) t -> p g t", p=P)[:, gsl, :],
            )
        nc.vector.memset(q8_t[:, :, STOK:], 0.0)           # rhs pad blocks
        nc.vector.memset(k8_t[:, :, :, 1, :], 0.0)         # lhsT zero blocks

        with (
            tc.tile_pool(name="psA", bufs=4, space="PSUM") as psA,
            tc.tile_pool(name="psQ", bufs=2, space="PSUM") as psQ,
        ):
            with tc.tile_pool(name="vT", bufs=1) as vTp:
                vT8 = vTp.tile([P, G, N2], F8, tag="vT8")

                nc.sync.dma_start(
                    out=vT8, in_=vT8d.ap().rearrange("(g p) n -> p g n", p=P)
                )
                with tc.tile_pool(name="ld", bufs=2) as ldp:
                    # ---- vv build: DR over cin pairs, bias ridden, /32 evict ----
                    for half in range(2):
                        wvt = wp.tile([P, 4, 2, TB], F8, name="wv", tag="w")
                        nc.sync.dma_start(out=wvt, in_=wkvv8.ap()[half])
                        pss = [
                            psA.tile([P, TB], FP, name="psv", tag="ps")
                            for _ in range(KB)
                        ]
                        for j in range(4):
                            for kb in range(KB):
                                nc.tensor.matmul(
                                    pss[kb],
                                    lhsT=vT8[:, 2 * j : 2 * j + 2, kb * P : (kb + 1) * P],
                                    rhs=wvt[:, j],
                                    start=(j == 0),
                                    stop=False,
                                    perf_mode=DRM,
                                )
                        for kb in range(KB):
                            nc.tensor.matmul(
                                pss[kb],
                                lhsT=ones8[0:1, :, :],
                                rhs=bkvv_t[0:1, half],
                                start=False,
                                stop=True,
                                perf_mode=DRM,
                            )
                            gs = slice(half * 4, half * 4 + 4)
                            ev().tensor_scalar_mul(
                                vv8[:, kb, gs, :],
                                pss[kb].rearrange("p (a j) -> p a j", j=SEG),
                                1.0 / WS,
                            )

                    # ---- kT: DR over cin pairs -> k8 (+bias, fp8) ----
                    for g in range(G):
                        wkt = wp.tile([P, 4, 2, P], F8, name="wk", tag="w")
                        nc.sync.dma_start(out=wkt, in_=wkvk8.ap()[g])
                        psk = psA.tile([P, N2], FP, name="psk", tag="ps")
                        for j in range(4):
                            nc.tensor.matmul(
                                psk,
                                lhsT=wkt[:, j],
                                rhs=vT8[:, 2 * j : 2 * j + 2, :],
                                start=(j == 0),
                                stop=(j == 3),
                                perf_mode=DRM,
                            )
                        ev().tensor_scalar_add(
                            k8_t[:, g, :, 0, :],
                            psk.rearrange("p (kb c) -> p kb c", c=P),
                            bkvk_t[:, g : g + 1],
                        )

                with tc.tile_pool(name="ldx", bufs=2) as ldx:
                    # ---- adaLN: silu(t + c) @ wada + bada (bf16) ----
                    c_nat = ldx.tile([F, C], FP, name="cnat", tag="misc", bufs=1)
                    nc.sync.dma_start(out=_r(c_nat), in_=_r(cmat.ap()))
                    for g in range(G):
                        pt = psA.tile([P, F], FP, name="ptc", tag="ps")
                        nc.tensor.transpose(
                            _r(pt), _r(c_nat[:, g * P : (g + 1) * P]), _r(ident[0:F, 0:F])
                        )
                        nc.scalar.activation(
                            out=silu_t[:, g, :],
                            in_=pt,
                            func=AF.Silu,
                            bias=t_t[:, g : g + 1],
                            scale=1.0,
                        )

                # ---- q projection: DR over cin pairs -> q8 (+bias) ----
                for g in range(G):
                    wqt = wp.tile([P, 4, 2, P], F8, name="wq", tag="w")
                    nc.sync.dma_start(out=wqt, in_=wq8.ap()[g])
                    pst2 = [
                        psQ.tile([P, 2, TB], FP, name=f"psq{i}", tag="psq")
                        for i in range(2)
                    ]
                    for j in range(4):
                        for tb in range(NTB):
                            nc.tensor.matmul(
                                pst2[tb // 2][:, tb % 2, :],
                                lhsT=wqt[:, j],
                                rhs=x8_t[:, 2 * j : 2 * j + 2, tb * TB : (tb + 1) * TB],
                                start=(j == 0),
                                stop=(j == 3),
                                perf_mode=DRM,
                            )
                    for i in range(2):
                        if (2 * g + i) % 2 == 0:
                            nc.scalar.activation(
                                out=q8_t[:, g, i * 2 * TB : (i + 1) * 2 * TB].rearrange(
                                    "p (a n) -> p a n", n=TB
                                ),
                                in_=pst2[i],
                                func=AF.Identity,
                                bias=bq_t[:, g : g + 1],
                                scale=1.0,
                            )
                        else:
                            nc.vector.tensor_scalar_add(
                                q8_t[:, g, i * 2 * TB : (i + 1) * 2 * TB].rearrange(
                                    "p (a n) -> p a n", n=TB
                                ),
                                pst2[i],
                                bq_t[:, g : g + 1],
                            )

        # ---- attention, with prev-tb normalize/proj/LN interleaved ----
        with (
            tc.tile_pool(name="sh", bufs=2, space="PSUM") as shp,
            tc.tile_pool(name="ex", bufs=2) as exp_pool,
            tc.tile_pool(name="sq", bufs=2) as sqp,
            tc.tile_pool(name="qbf", bufs=1) as qbfp,
        ):
            qbf = qbfp.tile([P, G, TB], BF, tag="qbf")
            sh_tiles = {}

            def norm_chunk(tb, g, pool):
                """q8[:, g, tb] = qbf (raw attn out, bf16) * 64/denom."""
                tbs = slice(tb * TB, (tb + 1) * TB)
                bc_ps = pool.tile([P, TB], FP, name="bcp", tag="pj")
                nc.tensor.matmul(
                    bc_ps,
                    lhsT=selp_t[:, g, :, :],
                    rhs=dnb16,
                    start=True,
                    stop=True,
                    perf_mode=DRM,
                )
                nc.vector.tensor_mul(q8_t[:, g, tbs], qbf[:, g, :], bc_ps)

            def proj_chunk(tb, g, pool):
                tbs = slice(tb * TB, (tb + 1) * TB)
                wpt = wp.tile([P, 4, 2, P], F8, name="wp8", tag="w")
                nc.sync.dma_start(out=wpt, in_=wproj8.ap()[g])
                pst = pool.tile([P, TB], FP, name="psp", tag="pj")
                for j in range(4):
                    nc.tensor.matmul(
                        pst,
                        lhsT=wpt[:, j],
                        rhs=q8_t[:, 2 * j : 2 * j + 2, tbs],
                        start=(j == 0),
                        stop=(j == 3),
                        perf_mode=DRM,
                    )
                nc.vector.scalar_tensor_tensor(
                    out=xT_bf[:, g, tbs],
                    in0=pst,
                    scalar=PROJ_SCALE,
                    in1=xT_bf[:, g, tbs],
                    op0=OP.mult,
                    op1=OP.add,
                )

            def ada_chunk(ct, pool):
                pa = pool.tile([P, TB], FP, name="pta", tag="pj")
                for ci in range(G):
                    nc.tensor.matmul(
                        pa[:, 0:F],
                        lhsT=watall[:, ct, ci, :],
                        rhs=silu_t[:, ci, :],
                        start=(ci == 0),
                        stop=(ci == G - 1),
                    )
                if ct < 8:
                    nc.vector.tensor_scalar_add(
                        ada_t[:, ct, :], pa[:, 0:F], bada_t[:, ct : ct + 1]
                    )
                else:
                    nc.vector.tensor_scalar(
                        ada_t[:, ct, :],
                        pa[:, 0:F],
                        bada_t[:, ct : ct + 1],
                        1.0,
                        op0=OP.add,
                        op1=OP.add,
                    )

            def ln_chunk(tb, g, sh, act_sq=False):
                """Accumulate sum(x1) into sh row 32, sum(x1^2) into row 64."""
                tbs = slice(tb * TB, (tb + 1) * TB)
                sqt = sqp.tile([P, TB], BF, tag="sq")
                if act_sq:
                    nc.scalar.activation(
                        out=sqt, in_=xT_bf[:, g, tbs], func=AF.Square, scale=1.0
                    )
                else:
                    nc.vector.tensor_mul(sqt, xT_bf[:, g, tbs], xT_bf[:, g, tbs])
                nc.tensor.matmul(
                    sh[32:33, :],
                    lhsT=ones_bf[:, 0:1],
                    rhs=xT_bf[:, g, tbs],
                    start=(g == 0),
                    stop=(g == G - 1),
                )
                nc.tensor.matmul(
                    sh[64:65, :],
                    lhsT=ones_bf[:, 0:1],
                    rhs=sqt,
                    start=(g == 0),
                    stop=(g == G - 1),
                )
                if g == G - 1:
                    mu = mu_all[0:1, tb, :]
                    nc.vector.tensor_scalar_mul(_r(mu), sh[32:33, :], 1.0 / C)
                    musq = sqp.tile([1, TB], FP, name="musq", tag="mu2")
                    nc.vector.tensor_mul(musq, mu, mu)
                    nc.vector.scalar_tensor_tensor(
                        out=var_all[0:1, tb, :],
                        in0=sh[64:65, :],
                        scalar=1.0 / C,
                        in1=musq,
                        op0=OP.mult,
                        op1=OP.subtract,
                    )

            with (
                tc.tile_pool(name="sc", bufs=2, space="PSUM") as scp,
                tc.tile_pool(name="ao", bufs=1, space="PSUM") as aop,
                tc.tile_pool(name="pj", bufs=1, space="PSUM") as pjp,
            ):
                for tb in range(NTB):
                    if tb == 0:
                        for ci4 in range(4):
                            nc.sync.dma_start(
                                out=watall[:, 4 * ci4 : 4 * ci4 + 4, :, :],
                                in_=wada.ap()[4 * ci4 : 4 * ci4 + 4].rearrange(
                                    "c p g k -> p c g k"
                                ),
                            )
                        for gi in range(4):
                            gsl = slice(2 * gi, 2 * gi + 2)
                            nc.sync.dma_start(
                                out=xT_bf[:, gsl, :],
                                in_=xTbf.ap().rearrange(
                                    "(g p) t -> p g t", p=P
                                )[:, gsl, :],
                            )
                    sh = shp.tile([P, TB], FP, name="shb", tag="sh")
                    sh_tiles[tb] = sh
                    for g in range(G):
                        ao_pair = aop.tile([P, TB], FP, name="aop", tag="ao")
                        # odd half first: its DR lhsT must span the full 128
                        # out partitions (DR rejects column tile offsets);
                        # the even half then re-zeroes rows 0:63.
                        for si, half in enumerate((1, 0)):
                            h2 = 2 * g + half
                            r0 = half * 64
                            ex8 = exp_pool.tile([P, KB, TB], F8, tag="ex")
                            rhs_q = q8_t[
                                r0 : r0 + 64, g, tb * TB : (tb + 2) * TB
                            ].rearrange("p (two n) -> p two n", two=2)
                            for jp in range(2):
                                sc = scp.tile([P, 2, TB], FP, name="scs", tag="sc")
                                for kk in range(2):
                                    kb = 2 * jp + kk
                                    nc.tensor.matmul(
                                        sc[:, kk, :],
                                        lhsT=k8_t[r0 : r0 + 64, g, kb, :, :],
                                        rhs=rhs_q,
                                        start=True,
                                        stop=True,
                                        perf_mode=DRM,
                                    )
                                nc.scalar.activation(
                                    out=ex8[:, 2 * jp : 2 * jp + 2, :],
                                    in_=sc,
                                    func=AF.Exp,
                                    scale=EXP_SCALE,
                                )
                            if half == 1:
                                ao_out, ao_lo, ao_hi = ao_pair, 0, P
                            else:
                                ao_out, ao_lo, ao_hi = ao_pair[0:64, :], 0, 64
                            for jp in range(2):
                                nc.tensor.matmul(
                                    ao_out,
                                    lhsT=vv8[:, 2 * jp : 2 * jp + 2, g, ao_lo:ao_hi],
                                    rhs=ex8[:, 2 * jp : 2 * jp + 2, :],
                                    start=(jp == 0),
                                    stop=(jp == 1),
                                    perf_mode=DRM,
                                )
                                nc.tensor.matmul(
                                    sh[0:16, :],
                                    lhsT=selc_t[:, h2, :, :],
                                    rhs=ex8[:, 2 * jp : 2 * jp + 2, :],
                                    start=(g == 0 and si == 0 and jp == 0),
                                    stop=(g == G - 1 and si == 1 and jp == 1),
                                    perf_mode=DRM,
                                )
                            # prev-tb chunks ride the ACT-bound head slots
                            slot = 2 * g + si
                            if tb > 0:
                                if slot < 8:
                                    norm_chunk(tb - 1, slot, pjp)
                                else:
                                    proj_chunk(tb - 1, slot - 8, pjp)
                                    ln_chunk(tb - 1, slot - 8, sh)
                            elif slot >= 8:
                                ada_chunk(2 * (slot - 8), pjp)
                                ada_chunk(2 * (slot - 8) + 1, pjp)
                        # stage raw attn-out pair (denominator not yet known)
                        nc.vector.tensor_copy(out=qbf[:, g, :], in_=ao_pair)
                    # all 32 collector matmuls done: one reciprocal per tb
                    nc.vector.reciprocal(out=dnb16[:, 0, :], in_=sh[0:16, :])

            # tail: overlap last-tb normalize/proj/LN with the f-loop for
            # the first three token blocks (their LN stats are final); the
            # batched sqrt is split 3+1 so only f6/f7 wait on the last tb.
            with (
                tc.tile_pool(name="psT", bufs=3, space="PSUM") as psT,
                tc.tile_pool(name="psD", bufs=3, space="PSUM") as psD,
                tc.tile_pool(name="st", bufs=2) as stp,
                tc.tile_pool(name="w1", bufs=2) as w1p,
                tc.tile_pool(name="rows", bufs=4) as rowp,
                tc.tile_pool(name="yo", bufs=2) as yop,
            ):
                def stats_block(tb_lo, tb_hi):
                    n = tb_hi - tb_lo
                    nc.scalar.activation(
                        out=_r(std_all[0:1, tb_lo:tb_hi, :].rearrange(
                            "one a n -> one (a n)"
                        )),
                        in_=var_all[0:1, tb_lo:tb_hi, :].rearrange(
                            "one a n -> one (a n)"
                        ),
                        func=AF.Sqrt,
                        bias=eps_t[0:1, :],
                        scale=1.0,
                    )
                    nc.vector.reciprocal(
                        out=_r(rst_all[0:1, tb_lo:tb_hi, :].rearrange(
                            "one a n -> one (a n)"
                        )),
                        in_=std_all[0:1, tb_lo:tb_hi, :].rearrange(
                            "one a n -> one (a n)"
                        ),
                    )

                def f_block(tb):
                    for f2 in range(2):
                        f = tb * 2 + f2
                        fcs = slice(f2 * NT, (f2 + 1) * NT)
                        gcs = slice(tb * TB + f2 * NT, tb * TB + (f2 + 1) * NT)
                        w1 = w1p.tile([P, G, OUTD], BF, tag="w1")
                        for g in range(G):
                            nc.vector.tensor_scalar_mul(
                                w1[:, g, :],
                                wlin_sb[:, g, :],
                                ada_t[:, 8 + g, f : f + 1],
                            )
                        ws1_ps = psD.tile([1, OUTD], FP, name="ws1", tag="ps")
                        c2_ps = psD.tile([1, OUTD], FP, name="c2", tag="ps")
                        for g in range(G):
                            nc.tensor.matmul(
                                ws1_ps,
                                lhsT=ada_bf[:, 8 + g, f : f + 1],
                                rhs=wlin_sb[:, g, :],
                                start=(g == 0),
                                stop=(g == G - 1),
                            )
                            nc.tensor.matmul(
                                c2_ps,
                                lhsT=ada_bf[:, g, f : f + 1],
                                rhs=wlin_sb[:, g, :],
                                start=(g == 0),
                                stop=(g == G - 1),
                            )
                        ws1n = rowp.tile([1, OUTD], FP, name="ws1n", tag="rows")
                        c2b = rowp.tile([1, OUTD], FP, name="c2b", tag="rows")
                        nc.vector.tensor_scalar_mul(_r(ws1n), ws1_ps, -1.0)
                        nc.vector.tensor_tensor(_r(c2b), c2_ps, blin_row, OP.add)
                        bc32_ps = psD.tile([OUTD, NT], FP, name="bc32", tag="ps")
                        nc.tensor.matmul(
                            bc32_ps,
                            lhsT=_r(ones_f),
                            rhs=_r(rst_all[0:1, tb, fcs]),
                            start=True,
                            stop=True,
                        )
                        bc32_sb = yop.tile([OUTD, NT], FP, name="bc32", tag="bc32")
                        nc.vector.tensor_copy(out=bc32_sb, in_=bc32_ps)
                        y_ps = psD.tile([OUTD, NT], FP, name="yps", tag="ps")
                        for g in range(G):
                            nc.tensor.matmul(
                                y_ps,
                                lhsT=w1[:, g, :],
                                rhs=xT_bf[:, g, gcs],
                                start=(g == 0),
                                stop=False,
                            )
                        nc.tensor.matmul(
                            y_ps,
                            lhsT=_r(ws1n),
                            rhs=_r(mu_all[0:1, tb, fcs]),
                            start=False,
                            stop=False,
                        )
                        nc.tensor.matmul(
                            y_ps,
                            lhsT=_r(c2b),
                            rhs=_r(std_all[0:1, tb, fcs]),
                            start=False,
                            stop=True,
                        )
                        yt = yop.tile([OUTD, NT], FP, tag="y")
                        nc.vector.tensor_mul(yt, y_ps, bc32_sb)
                        nc.sync.dma_start(out=yT[:, gcs], in_=yt)

                nc.vector.tensor_copy(out=ada_bf, in_=ada_t)
                stats_block(0, NTB - 1)
                for g in range(G):
                    norm_chunk(NTB - 1, g, psT)
                for g in range(G):
                    proj_chunk(NTB - 1, g, psT)
                for g in range(G):
                    ln_chunk(NTB - 1, g, sh_tiles[NTB - 1], act_sq=True)
                for tb in range(NTB - 1):
                    f_block(tb)
                stats_block(NTB - 1, NTB)
                f_block(NTB - 1)


def declare_io(nc):
    return {
        "f8b": nc.dram_tensor("f8b", [_F8_TOT], F8, kind="ExternalInput"),
        "bfb": nc.dram_tensor("bfb", [_BF_TOT], BF, kind="ExternalInput"),
        "f32b": nc.dram_tensor("f32b", [_F32_TOT], FP, kind="ExternalInput"),
        "yT": nc.dram_tensor("yT", [OUTD, STOK], FP, kind="ExternalOutput"),
    }


class _Sect:
    """dram-blob section handle mimicking a dram tensor (.ap() slicing)."""

    def __init__(self, blob, off, n, shape):
        self._blob, self._off, self._n, self._shape = blob, off, n, shape

    def ap(self):
        flat = self._blob.ap()[self._off : self._off + self._n]
        dims = self._shape
        names = " ".join(f"d{i}" for i in range(len(dims)))
        kw = {f"d{i}": d for i, d in enumerate(dims)}
        return flat.rearrange(f"({names}) -> {names}", **kw)


def _sections(io):
    out = {}
    for name, (off, n, shape) in _F8_OFF.items():
        out[name] = _Sect(io["f8b"], off, n, shape)
    for name, (off, n, shape) in _BF_OFF.items():
        out[name] = _Sect(io["bfb"], off, n, shape)
    for name, (off, n, shape) in _F32_OFF.items():
        out[name] = _Sect(io["f32b"], off, n, shape)
    out["yT"] = io["yT"]
    return out


def build_nc():
    nc = bacc.Bacc("TRN2", target_bir_lowering=False, debug=False)
    io = declare_io(nc)
    with tile.TileContext(nc) as tc:
        _body(nc, tc, io)
    nc.compile()
    return nc


_CACHE = {}


def _get_nc():
    if "nc" not in _CACHE:
        _CACHE["nc"] = build_nc()
    return _CACHE["nc"]


def _block_w(w):
    """[cin=1024, cout] -> [cout_grp, p, j, b, cout_col] for DR lhsT tiles."""
    co = w.shape[1] // P
    return np.ascontiguousarray(
        w.reshape(4, 2, P, co, P).transpose(3, 2, 0, 1, 4)
    )


def make_in_maps(x, v, t, c, wq, bq, wkv, bkv, wproj, bproj, wada, bada, wlin, blin):
    f32 = lambda a: np.ascontiguousarray(np.asarray(a, dtype=np.float32))
    x, v, t, c = f32(x), f32(v), f32(t), f32(c)
    wq, wkv, wproj, wada = f32(wq), f32(wkv), f32(wproj), f32(wada)
    bq, bkv, bproj, bada = f32(bq), f32(bkv), f32(bproj), f32(bada)
    wlin, blin = f32(wlin), f32(blin)

    wkvv = np.ascontiguousarray(wkv[:, C:])
    wkvv8 = np.ascontiguousarray(
        (WS * wkvv).reshape(4, 2, P, 2, TB).transpose(3, 2, 0, 1, 4)
    ).astype(NP_F8)
    bkvv8 = np.zeros((1, 2, 2, TB), np.float32)
    bkvv8[0, :, 0, :] = (WS * bkv[C:]).reshape(2, TB)
    wada_b = np.ascontiguousarray(
        wada.reshape(G, P, 16, P).transpose(2, 1, 0, 3)
    ).astype(NP_BF)
    selc = np.zeros((P, 16, 2, 16), np.float32)
    for h in range(16):
        selc[:, h, :, h] = ONES_COL
    selp = np.zeros((16, G, 2, P), np.float32)
    for g in range(G):
        selp[2 * g, g, 0, 0:64] = 1.0
        selp[2 * g + 1, g, 0, 64:128] = 1.0

    f8_shared = {
        "wq8": _block_w(WS * wq).astype(NP_F8),
        "wkvk8": _block_w(WS * wkv[:, :C]).astype(NP_F8),
        "wkvv8": wkvv8.astype(NP_F8),
        "wproj8": _block_w(WS * wproj).astype(NP_F8),
        "bkvv8": bkvv8.astype(NP_F8),
        "selc8": selc.astype(NP_F8),
        "selp8": selp.astype(NP_F8),
    }
    bf_shared = {
        "wada_bf": wada_b,
        "wlin_bf": wlin.astype(NP_BF),
    }

    def pack(sects, parts, np_dt):
        flats = []
        for name, (off, n, shape) in sects.items():
            a = parts[name]
            assert a.size == n, (name, a.shape, shape)
            flats.append(np.ascontiguousarray(a).reshape(-1))
        return np.concatenate(flats).astype(np_dt, copy=False)

    in_maps = []
    vT8_cache = {}
    for m in range(8):
        b, half = divmod(m, 2)
        xs = x[b, half * STOK : (half + 1) * STOK, :]
        if b not in vT8_cache:
            vT8_cache[b] = np.ascontiguousarray(v[b].T).astype(NP_F8)
        f8_parts = {
            "xT8": np.ascontiguousarray(xs.T).astype(NP_F8),
            "vT8d": vT8_cache[b],
            **f8_shared,
        }
        bf_parts = {
            "xTbf": np.ascontiguousarray((xs + bproj[None, :]).T).astype(NP_BF),
            **bf_shared,
        }
        f32_parts = {
            "t_b": f32(t[b]),
            "c_sl": f32(c[b, half * F : (half + 1) * F, :]),
            "bq32": f32(WS * bq),
            "bkvk32": f32(WS * bkv[:C]),
            "bada": f32(bada),
            "blin": f32(blin),
        }
        in_maps.append(
            {
                "f8b": pack(_F8_OFF, f8_parts, NP_F8),
                "bfb": pack(_BF_OFF, bf_parts, NP_BF),
                "f32b": pack(_F32_OFF, f32_parts, np.float32),
            }
        )
    return in_maps


def assemble_y(results):
    y = np.empty((B, T, NT, OUTD), np.float32)
    for m in range(8):
        b, half = divmod(m, 2)
        yt = np.asarray(results[m]["yT"])  # [OUTD, STOK]
        y[b, half * F : (half + 1) * F] = yt.T.reshape(F, NT, OUTD)
    return y


def kernel(x, v, t, c, wq, bq, wkv, bkv, wproj, bproj, wada, bada, wlin, blin, T=16, H=16):
    nc = _get_nc()
    in_maps = make_in_maps(
        x, v, t, c, wq, bq, wkv, bkv, wproj, bproj, wada, bada, wlin, blin
    )
    res = run_bass_kernel_spmd(nc, in_maps, core_ids=list(range(8)))
    return assemble_y(res.results)


# revision 4
# speedup vs baseline: 11.5692x; 2.4966x over previous
"""Trainium2 Bass kernel: cross-attention + adaLN-LN + linear block, fp8/bf16.

Sharding: 8 cores = 4 batches x 2 token-halves of S=4096 (as baseline).

Key differences from the fp32r baseline:
- All large GEMMs run as fp8e4m3 DoubleRow matmuls: two K=128 blocks packed
  side-by-side in the free dim of both operands, halving PE time per MAC.
  Contractions over C pair adjacent cin groups; attention scores use a
  zeroed second lhsT block (K=64 real); attn-out pairs adjacent key blocks.
- Weights are host-scaled by 32 so fp8 values sit in e4m3's normal range;
  the exp activation scale absorbs 32*32, the proj eviction absorbs 2^-11,
  and the softmax ones-column is 1/64 so reciprocals (64/denom ~ 0.09) stay
  normal in fp8.
- x/x1 master is bf16; LN sums, adaLN and the final linear run in bf16.
- Softmax: scores for a kb-pair land in a 2-bank PSUM tile and one Exp
  activation (free size 1024) converts them straight to fp8.
- Eviction work is split between the DVE and GpSimd engines; proj and LN
  chunks of the previous token block are interleaved between attention
  heads so the PE fills the gaps while the ACT engine streams exps.
- LN sqrt is deferred to one batched activation to avoid ACT table thrash.
"""

import sys

for _p in ("/opt/trn_rl_repo", "/opt/pypackages"):
    if _p not in sys.path:
        sys.path.append(_p)

import numpy as np
import ml_dtypes

import concourse.bacc as bacc
import concourse.tile as tile
from concourse import mybir
from concourse.bass_utils import run_bass_kernel_spmd
from concourse.masks import make_identity

FP = mybir.dt.float32
FPR = mybir.dt.float32r
BF = mybir.dt.bfloat16
F8 = mybir.dt.float8e4
AF = mybir.ActivationFunctionType
OP = mybir.AluOpType
DRM = mybir.MatmulPerfMode.DoubleRow

NP_BF = ml_dtypes.bfloat16
NP_F8 = ml_dtypes.float8_e4m3


def _r(ap):
    return ap.bitcast(FPR)


# Problem sizes (hardcoded per spec).
B = 4
S = 4096
C = 1024
N2 = 512
H = 16
D = 64
T = 16
NT = 256          # tokens per frame
OUTD = 32

STOK = S // 2     # tokens per core
F = 8             # frames per core
G = C // 128      # 8 channel groups
TB = 512          # token block
NTB = STOK // TB  # 4
KB = N2 // 128    # 4 key blocks
P = 128
SEG = 128         # vv cols per head-pair segment

WS = 32.0                      # host weight scale
EXP_SCALE = (D ** -0.5) / (WS * WS)
ONES_COL = 1.0 / 64.0          # softmax denom ride scale
PROJ_SCALE = 1.0 / (64.0 * WS)  # ao8 = 64*ao, wproj8 = 32*wproj
EPS = 1e-6

# ---- packed input blobs: one dram tensor per dtype to minimize the
# per-input dispatch overhead (measured ~40us/input through this stack) ----
_F8_SECTS = [
    ("xT8", (C, STOK)),
    ("vT8d", (C, N2)),
    ("wq8", (G, P, 4, 2, P)),
    ("wkvk8", (G, P, 4, 2, P)),
    ("wkvv8", (2, P, 4, 2, TB)),
    ("wproj8", (G, P, 4, 2, P)),
    ("bkvv8", (1, 2, 2, TB)),
    ("selc8", (P, 16, 2, 16)),
    ("selp8", (16, G, 2, P)),
]
_BF_SECTS = [
    ("xTbf", (C, STOK)),
    ("wada_bf", (16, P, G, P)),
    ("wlin_bf", (C, OUTD)),
]
_F32_SECTS = [
    ("t_b", (C,)),
    ("c_sl", (F, C)),
    ("bq32", (C,)),
    ("bkvk32", (C,)),
    ("bada", (2 * C,)),
    ("blin", (OUTD,)),
]


def _offsets(sects):
    out, off = {}, 0
    for name, shape in sects:
        n = int(np.prod(shape))
        out[name] = (off, n, shape)
        off += n
    return out, off


_F8_OFF, _F8_TOT = _offsets(_F8_SECTS)
_BF_OFF, _BF_TOT = _offsets(_BF_SECTS)
_F32_OFF, _F32_TOT = _offsets(_F32_SECTS)
_BF_BASE = _F8_TOT                      # byte offsets into the single blob
_F32_BASE = _BF_BASE + 2 * _BF_TOT
_BLOB_BYTES = _F32_BASE + 4 * _F32_TOT


def _body(nc, tc, io):
    with nc.allow_low_precision("fp8/bf16 matmul operands"):
        _body_inner(nc, tc, io)


def _body_inner(nc, tc, io):
    io = _sections(io)
    xTbf, xT8d, vT8d = io["xTbf"], io["xT8"], io["vT8d"]
    tvec, cmat = io["t_b"], io["c_sl"]
    wq8, bq32 = io["wq8"], io["bq32"]
    wkvk8, wkvv8, bkvk32, bkvv8 = io["wkvk8"], io["wkvv8"], io["bkvk32"], io["bkvv8"]
    wproj8 = io["wproj8"]
    wada, bada = io["wada_bf"], io["bada"]
    wlin, blin = io["wlin_bf"], io["blin"]
    yT = io["yT"]

    def ev():
        return nc.vector  # all PSUM-reading evictions must be on DVE

    with (
        tc.tile_pool(name="consts", bufs=1) as consts,
        tc.tile_pool(name="xT", bufs=1) as xTp,
        tc.tile_pool(name="x8", bufs=1) as x8p,
        tc.tile_pool(name="q8", bufs=1) as q8p,
        tc.tile_pool(name="k8", bufs=1) as k8p,
        tc.tile_pool(name="vv", bufs=1) as vvp,
        tc.tile_pool(name="wp", bufs=4) as wp,
    ):
        # ---- constants ----
        scratch = consts.tile([P, P], FP, tag="scratch")
        make_identity(nc, scratch)
        ident = consts.tile([P, P], FP, tag="ident")
        nc.vector.tensor_copy(out=_r(ident), in_=scratch)
        ones8 = consts.tile([P, 2, P], F8, tag="ones8")
        nc.vector.memset(ones8[:, 0, :], 1.0)
        nc.vector.memset(ones8[:, 1, :], 0.0)
        ones_bf = consts.tile([P, 2], BF, tag="onesbf")
        nc.vector.memset(ones_bf, 1.0)
        ones_f = consts.tile([1, OUTD], FP, tag="onesf")
        nc.vector.memset(ones_f, 1.0)
        eps_t = consts.tile([1, 1], FP, tag="eps")
        nc.vector.memset(eps_t, EPS)

        bq_t = consts.tile([P, G], FP, tag="bq")
        nc.sync.dma_start(out=bq_t, in_=bq32.ap().rearrange("(g p) -> p g", p=P))
        bkvk_t = consts.tile([P, G], FP, tag="bkvk")
        nc.sync.dma_start(out=bkvk_t, in_=bkvk32.ap().rearrange("(g p) -> p g", p=P))
        bkvv_t = consts.tile([1, 2, 2, TB], F8, tag="bkvv")
        nc.sync.dma_start(out=bkvv_t, in_=bkvv8.ap())
        bada_t = consts.tile([P, 16], FP, tag="bada")
        nc.sync.dma_start(out=bada_t, in_=bada.ap().rearrange("(g p) -> p g", p=P))
        blin_row = consts.tile([1, OUTD], FP, tag="blin")
        nc.sync.dma_start(
            out=blin_row, in_=blin.ap().rearrange("(one o) -> one o", one=1)
        )
        t_t = consts.tile([P, G], FP, tag="tvec")
        nc.sync.dma_start(out=t_t, in_=tvec.ap().rearrange("(g p) -> p g", p=P))
        wlin_sb = consts.tile([P, G, OUTD], BF, tag="wlin")
        nc.sync.dma_start(
            out=wlin_sb, in_=wlin.ap().rearrange("(ci p) o -> p ci o", p=P)
        )
        watall = consts.tile([P, 16, G, P], BF, tag="watall")
        silu_t = consts.tile([P, G, F], BF, tag="silu")
        ada_t = consts.tile([P, 16, F], FP, tag="ada")
        ada_bf = consts.tile([P, 16, F], BF, tag="adabf")
        selc_t = consts.tile([P, 16, 2, 16], F8, tag="selc")
        nc.sync.dma_start(out=selc_t, in_=io["selc8"].ap())
        selp_t = consts.tile([16, G, 2, P], F8, tag="selp")
        nc.sync.dma_start(out=selp_t, in_=io["selp8"].ap())
        dnb16 = consts.tile([16, 2, TB], F8, tag="dnb16")
        nc.vector.memset(dnb16, 0.0)
        mu_all = consts.tile([1, NTB, TB], FP, tag="mu")
        var_all = consts.tile([1, NTB, TB], FP, tag="var")
        std_all = consts.tile([1, NTB, TB], FP, tag="std")
        rst_all = consts.tile([1, NTB, TB], FP, tag="rst")

        # ---- persistent activations (hosts supplies transposed x) ----
        xT_bf = xTp.tile([P, G, STOK], BF, tag="xT")
        x8_t = x8p.tile([P, G, STOK], F8, tag="x8")
        q8_t = q8p.tile([P, G, STOK + TB], F8, tag="q8")   # +pad for rhs blocks
        k8_t = k8p.tile([P, G, KB, 2, P], F8, tag="k8")
        vv8 = vvp.tile([P, KB, G, SEG], F8, tag="vv8")

        # zero-fill the regions matmuls read but evictions never write
        for gi in range(4):
            gsl = slice(2 * gi, 2 * gi + 2)
            nc.gpsimd.dma_start(
                out=x8_t[:, gsl, :],
                in_=xT8d.ap().rearrange("(g p) t -> p g t", p=P)[:, gsl, :],
            )
        nc.vector.memset(q8_t[:, :, STOK:], 0.0)           # rhs pad blocks
        nc.vector.memset(k8_t[:, :, :, 1, :], 0.0)         # lhsT zero blocks

        with (
            tc.tile_pool(name="psA", bufs=4, space="PSUM") as psA,
            tc.tile_pool(name="psQ", bufs=2, space="PSUM") as psQ,
        ):
            with tc.tile_pool(name="vT", bufs=1) as vTp:
                vT8 = vTp.tile([P, G, N2], F8, tag="vT8")

                nc.sync.dma_start(
                    out=vT8, in_=vT8d.ap().rearrange("(g p) n -> p g n", p=P)
                )
                with tc.tile_pool(name="ld", bufs=2) as ldp:
                    # ---- vv build: DR over cin pairs, bias ridden, /32 evict ----
                    for half in range(2):
                        wvt = wp.tile([P, 4, 2, TB], F8, name="wv", tag="w")
                        nc.sync.dma_start(out=wvt, in_=wkvv8.ap()[half])
                        pss = [
                            psA.tile([P, TB], FP, name="psv", tag="ps")
                            for _ in range(KB)
                        ]
                        for j in range(4):
                            for kb in range(KB):
                                nc.tensor.matmul(
                                    pss[kb],
                                    lhsT=vT8[:, 2 * j : 2 * j + 2, kb * P : (kb + 1) * P],
                                    rhs=wvt[:, j],
                                    start=(j == 0),
                                    stop=False,
                                    perf_mode=DRM,
                                )
                        for kb in range(KB):
                            nc.tensor.matmul(
                                pss[kb],
                                lhsT=ones8[0:1, :, :],
                                rhs=bkvv_t[0:1, half],
                                start=False,
                                stop=True,
                                perf_mode=DRM,
                            )
                            gs = slice(half * 4, half * 4 + 4)
                            ev().tensor_scalar_mul(
                                vv8[:, kb, gs, :],
                                pss[kb].rearrange("p (a j) -> p a j", j=SEG),
                                1.0 / WS,
                            )

                    # ---- kT: DR over cin pairs -> k8 (+bias, fp8) ----
                    for g in range(G):
                        wkt = wp.tile([P, 4, 2, P], F8, name="wk", tag="w")
                        nc.sync.dma_start(out=wkt, in_=wkvk8.ap()[g])
                        psk = psA.tile([P, N2], FP, name="psk", tag="ps")
                        for j in range(4):
                            nc.tensor.matmul(
                                psk,
                                lhsT=wkt[:, j],
                                rhs=vT8[:, 2 * j : 2 * j + 2, :],
                                start=(j == 0),
                                stop=(j == 3),
                                perf_mode=DRM,
                            )
                        ev().tensor_scalar_add(
                            k8_t[:, g, :, 0, :],
                            psk.rearrange("p (kb c) -> p kb c", c=P),
                            bkvk_t[:, g : g + 1],
                        )

                with tc.tile_pool(name="ldx", bufs=2) as ldx:
                    # ---- adaLN: silu(t + c) @ wada + bada (bf16) ----
                    c_nat = ldx.tile([F, C], FP, name="cnat", tag="misc", bufs=1)
                    nc.sync.dma_start(out=_r(c_nat), in_=_r(cmat.ap()))
                    for g in range(G):
                        pt = psA.tile([P, F], FP, name="ptc", tag="ps")
                        nc.tensor.transpose(
                            _r(pt), _r(c_nat[:, g * P : (g + 1) * P]), _r(ident[0:F, 0:F])
                        )
                        nc.scalar.activation(
                            out=silu_t[:, g, :],
                            in_=pt,
                            func=AF.Silu,
                            bias=t_t[:, g : g + 1],
                            scale=1.0,
                        )

                # ---- q projection: DR over cin pairs -> q8 (+bias) ----
                for g in range(G):
                    wqt = wp.tile([P, 4, 2, P], F8, name="wq", tag="w")
                    nc.sync.dma_start(out=wqt, in_=wq8.ap()[g])
                    pst2 = [
                        psQ.tile([P, 2, TB], FP, name=f"psq{i}", tag="psq")
                        for i in range(2)
                    ]
                    for j in range(4):
                        for tb in range(NTB):
                            nc.tensor.matmul(
                                pst2[tb // 2][:, tb % 2, :],
                                lhsT=wqt[:, j],
                                rhs=x8_t[:, 2 * j : 2 * j + 2, tb * TB : (tb + 1) * TB],
                                start=(j == 0),
                                stop=(j == 3),
                                perf_mode=DRM,
                            )
                    for i in range(2):
                        if (2 * g + i) % 2 == 0:
                            nc.scalar.activation(
                                out=q8_t[:, g, i * 2 * TB : (i + 1) * 2 * TB].rearrange(
                                    "p (a n) -> p a n", n=TB
                                ),
                                in_=pst2[i],
                                func=AF.Identity,
                                bias=bq_t[:, g : g + 1],
                                scale=1.0,
                            )
                        else:
                            nc.vector.tensor_scalar_add(
                                q8_t[:, g, i * 2 * TB : (i + 1) * 2 * TB].rearrange(
                                    "p (a n) -> p a n", n=TB
                                ),
                                pst2[i],
                                bq_t[:, g : g + 1],
                            )

        # ---- attention, with prev-tb normalize/proj/LN interleaved ----
        with (
            tc.tile_pool(name="sh", bufs=2, space="PSUM") as shp,
            tc.tile_pool(name="ex", bufs=2) as exp_pool,
            tc.tile_pool(name="sq", bufs=2) as sqp,
            tc.tile_pool(name="qbf", bufs=1) as qbfp,
        ):
            qbf = qbfp.tile([P, G, TB], BF, tag="qbf")
            sh_tiles = {}

            def norm_chunk(tb, g, pool):
                """q8[:, g, tb] = qbf (raw attn out, bf16) * 64/denom."""
                tbs = slice(tb * TB, (tb + 1) * TB)
                bc_ps = pool.tile([P, TB], FP, name="bcp", tag="pj")
                nc.tensor.matmul(
                    bc_ps,
                    lhsT=selp_t[:, g, :, :],
                    rhs=dnb16,
                    start=True,
                    stop=True,
                    perf_mode=DRM,
                )
                nc.vector.tensor_mul(q8_t[:, g, tbs], qbf[:, g, :], bc_ps)

            def proj_chunk(tb, g, pool):
                tbs = slice(tb * TB, (tb + 1) * TB)
                wpt = wp.tile([P, 4, 2, P], F8, name="wp8", tag="w")
                nc.sync.dma_start(out=wpt, in_=wproj8.ap()[g])
                pst = pool.tile([P, TB], FP, name="psp", tag="pj")
                for j in range(4):
                    nc.tensor.matmul(
                        pst,
                        lhsT=wpt[:, j],
                        rhs=q8_t[:, 2 * j : 2 * j + 2, tbs],
                        start=(j == 0),
                        stop=(j == 3),
                        perf_mode=DRM,
                    )
                nc.vector.scalar_tensor_tensor(
                    out=xT_bf[:, g, tbs],
                    in0=pst,
                    scalar=PROJ_SCALE,
                    in1=xT_bf[:, g, tbs],
                    op0=OP.mult,
                    op1=OP.add,
                )

            def ada_chunk(ct, pool):
                pa = pool.tile([P, TB], FP, name="pta", tag="pj")
                for ci in range(G):
                    nc.tensor.matmul(
                        pa[:, 0:F],
                        lhsT=watall[:, ct, ci, :],
                        rhs=silu_t[:, ci, :],
                        start=(ci == 0),
                        stop=(ci == G - 1),
                    )
                if ct < 8:
                    nc.vector.tensor_scalar_add(
                        ada_t[:, ct, :], pa[:, 0:F], bada_t[:, ct : ct + 1]
                    )
                else:
                    nc.vector.tensor_scalar(
                        ada_t[:, ct, :],
                        pa[:, 0:F],
                        bada_t[:, ct : ct + 1],
                        1.0,
                        op0=OP.add,
                        op1=OP.add,
                    )

            def ln_chunk(tb, g, sh, act_sq=False):
                """Accumulate sum(x1) into sh row 32, sum(x1^2) into row 64."""
                tbs = slice(tb * TB, (tb + 1) * TB)
                sqt = sqp.tile([P, TB], BF, tag="sq")
                if act_sq:
                    nc.scalar.activation(
                        out=sqt, in_=xT_bf[:, g, tbs], func=AF.Square, scale=1.0
                    )
                else:
                    nc.vector.tensor_mul(sqt, xT_bf[:, g, tbs], xT_bf[:, g, tbs])
                nc.tensor.matmul(
                    sh[32:33, :],
                    lhsT=ones_bf[:, 0:1],
                    rhs=xT_bf[:, g, tbs],
                    start=(g == 0),
                    stop=(g == G - 1),
                )
                nc.tensor.matmul(
                    sh[64:65, :],
                    lhsT=ones_bf[:, 0:1],
                    rhs=sqt,
                    start=(g == 0),
                    stop=(g == G - 1),
                )
                if g == G - 1:
                    mu = mu_all[0:1, tb, :]
                    nc.vector.tensor_scalar_mul(_r(mu), sh[32:33, :], 1.0 / C)
                    musq = sqp.tile([1, TB], FP, name="musq", tag="mu2")
                    nc.vector.tensor_mul(musq, mu, mu)
                    nc.vector.scalar_tensor_tensor(
                        out=var_all[0:1, tb, :],
                        in0=sh[64:65, :],
                        scalar=1.0 / C,
                        in1=musq,
                        op0=OP.mult,
                        op1=OP.subtract,
                    )

            with (
                tc.tile_pool(name="sc", bufs=2, space="PSUM") as scp,
                tc.tile_pool(name="ao", bufs=1, space="PSUM") as aop,
                tc.tile_pool(name="pj", bufs=1, space="PSUM") as pjp,
            ):
                for tb in range(NTB):
                    if tb == 0:
                        for ci4 in range(4):
                            nc.sync.dma_start(
                                out=watall[:, 4 * ci4 : 4 * ci4 + 4, :, :],
                                in_=wada.ap()[4 * ci4 : 4 * ci4 + 4].rearrange(
                                    "c p g k -> p c g k"
                                ),
                            )
                        for gi in range(4):
                            gsl = slice(2 * gi, 2 * gi + 2)
                            nc.sync.dma_start(
                                out=xT_bf[:, gsl, :],
                                in_=xTbf.ap().rearrange(
                                    "(g p) t -> p g t", p=P
                                )[:, gsl, :],
                            )
                    sh = shp.tile([P, TB], FP, name="shb", tag="sh")
                    sh_tiles[tb] = sh
                    for g in range(G):
                        ao_pair = aop.tile([P, TB], FP, name="aop", tag="ao")
                        # odd half first: its DR lhsT must span the full 128
                        # out partitions (DR rejects column tile offsets);
                        # the even half then re-zeroes rows 0:63.
                        for si, half in enumerate((1, 0)):
                            h2 = 2 * g + half
                            r0 = half * 64
                            ex8 = exp_pool.tile([P, KB, TB], F8, tag="ex")
                            rhs_q = q8_t[
                                r0 : r0 + 64, g, tb * TB : (tb + 2) * TB
                            ].rearrange("p (two n) -> p two n", two=2)
                            for jp in range(2):
                                sc = scp.tile([P, 2, TB], FP, name="scs", tag="sc")
                                for kk in range(2):
                                    kb = 2 * jp + kk
                                    nc.tensor.matmul(
                                        sc[:, kk, :],
                                        lhsT=k8_t[r0 : r0 + 64, g, kb, :, :],
                                        rhs=rhs_q,
                                        start=True,
                                        stop=True,
                                        perf_mode=DRM,
                                    )
                                nc.scalar.activation(
                                    out=ex8[:, 2 * jp : 2 * jp + 2, :],
                                    in_=sc,
                                    func=AF.Exp,
                                    scale=EXP_SCALE,
                                )
                            if half == 1:
                                ao_out, ao_lo, ao_hi = ao_pair, 0, P
                            else:
                                ao_out, ao_lo, ao_hi = ao_pair[0:64, :], 0, 64
                            for jp in range(2):
                                nc.tensor.matmul(
                                    ao_out,
                                    lhsT=vv8[:, 2 * jp : 2 * jp + 2, g, ao_lo:ao_hi],
                                    rhs=ex8[:, 2 * jp : 2 * jp + 2, :],
                                    start=(jp == 0),
                                    stop=(jp == 1),
                                    perf_mode=DRM,
                                )
                                nc.tensor.matmul(
                                    sh[0:16, :],
                                    lhsT=selc_t[:, h2, :, :],
                                    rhs=ex8[:, 2 * jp : 2 * jp + 2, :],
                                    start=(g == 0 and si == 0 and jp == 0),
                                    stop=(g == G - 1 and si == 1 and jp == 1),
                                    perf_mode=DRM,
                                )
                            # prev-tb chunks ride the ACT-bound head slots
                            slot = 2 * g + si
                            if tb > 0:
                                if slot < 8:
                                    norm_chunk(tb - 1, slot, pjp)
                                else:
                                    proj_chunk(tb - 1, slot - 8, pjp)
                                    ln_chunk(tb - 1, slot - 8, sh)
                            elif slot >= 8:
                                ada_chunk(2 * (slot - 8), pjp)
                                ada_chunk(2 * (slot - 8) + 1, pjp)
                        # stage raw attn-out pair (denominator not yet known)
                        nc.vector.tensor_copy(out=qbf[:, g, :], in_=ao_pair)
                    # all 32 collector matmuls done: one reciprocal per tb
                    nc.vector.reciprocal(out=dnb16[:, 0, :], in_=sh[0:16, :])

            # tail: overlap last-tb normalize/proj/LN with the f-loop for
            # the first three token blocks (their LN stats are final); the
            # batched sqrt is split 3+1 so only f6/f7 wait on the last tb.
            with (
                tc.tile_pool(name="psT", bufs=3, space="PSUM") as psT,
                tc.tile_pool(name="psD", bufs=3, space="PSUM") as psD,
                tc.tile_pool(name="st", bufs=2) as stp,
                tc.tile_pool(name="w1", bufs=2) as w1p,
                tc.tile_pool(name="rows", bufs=4) as rowp,
                tc.tile_pool(name="yo", bufs=2) as yop,
            ):
                def stats_block(tb_lo, tb_hi):
                    n = tb_hi - tb_lo
                    nc.scalar.activation(
                        out=_r(std_all[0:1, tb_lo:tb_hi, :].rearrange(
                            "one a n -> one (a n)"
                        )),
                        in_=var_all[0:1, tb_lo:tb_hi, :].rearrange(
                            "one a n -> one (a n)"
                        ),
                        func=AF.Sqrt,
                        bias=eps_t[0:1, :],
                        scale=1.0,
                    )
                    nc.vector.reciprocal(
                        out=_r(rst_all[0:1, tb_lo:tb_hi, :].rearrange(
                            "one a n -> one (a n)"
                        )),
                        in_=std_all[0:1, tb_lo:tb_hi, :].rearrange(
                            "one a n -> one (a n)"
                        ),
                    )

                def f_block(tb):
                    for f2 in range(2):
                        f = tb * 2 + f2
                        fcs = slice(f2 * NT, (f2 + 1) * NT)
                        gcs = slice(tb * TB + f2 * NT, tb * TB + (f2 + 1) * NT)
                        w1 = w1p.tile([P, G, OUTD], BF, tag="w1")
                        for g in range(G):
                            nc.vector.tensor_scalar_mul(
                                w1[:, g, :],
                                wlin_sb[:, g, :],
                                ada_t[:, 8 + g, f : f + 1],
                            )
                        ws1_ps = psD.tile([1, OUTD], FP, name="ws1", tag="ps")
                        c2_ps = psD.tile([1, OUTD], FP, name="c2", tag="ps")
                        for g in range(G):
                            nc.tensor.matmul(
                                ws1_ps,
                                lhsT=ada_bf[:, 8 + g, f : f + 1],
                                rhs=wlin_sb[:, g, :],
                                start=(g == 0),
                                stop=(g == G - 1),
                            )
                            nc.tensor.matmul(
                                c2_ps,
                                lhsT=ada_bf[:, g, f : f + 1],
                                rhs=wlin_sb[:, g, :],
                                start=(g == 0),
                                stop=(g == G - 1),
                            )
                        ws1n = rowp.tile([1, OUTD], FP, name="ws1n", tag="rows")
                        c2b = rowp.tile([1, OUTD], FP, name="c2b", tag="rows")
                        nc.vector.tensor_scalar_mul(_r(ws1n), ws1_ps, -1.0)
                        nc.vector.tensor_tensor(_r(c2b), c2_ps, blin_row, OP.add)
                        bc32_ps = psD.tile([OUTD, NT], FP, name="bc32", tag="ps")
                        nc.tensor.matmul(
                            bc32_ps,
                            lhsT=_r(ones_f),
                            rhs=_r(rst_all[0:1, tb, fcs]),
                            start=True,
                            stop=True,
                        )
                        bc32_sb = yop.tile([OUTD, NT], FP, name="bc32", tag="bc32")
                        nc.vector.tensor_copy(out=bc32_sb, in_=bc32_ps)
                        y_ps = psD.tile([OUTD, NT], FP, name="yps", tag="ps")
                        for g in range(G):
                            nc.tensor.matmul(
                                y_ps,
                                lhsT=w1[:, g, :],
                                rhs=xT_bf[:, g, gcs],
                                start=(g == 0),
                                stop=False,
                            )
                        nc.tensor.matmul(
                            y_ps,
                            lhsT=_r(ws1n),
                            rhs=_r(mu_all[0:1, tb, fcs]),
                            start=False,
                            stop=False,
                        )
                        nc.tensor.matmul(
                            y_ps,
                            lhsT=_r(c2b),
                            rhs=_r(std_all[0:1, tb, fcs]),
                            start=False,
                            stop=True,
                        )
                        yt = yop.tile([OUTD, NT], FP, tag="y")
                        nc.vector.tensor_mul(yt, y_ps, bc32_sb)
                        nc.sync.dma_start(out=yT[:, gcs], in_=yt)

                nc.vector.tensor_copy(out=ada_bf, in_=ada_t)
                stats_block(0, NTB - 1)
                for g in range(G):
                    norm_chunk(NTB - 1, g, psT)
                for g in range(G):
                    proj_chunk(NTB - 1, g, psT)
                for g in range(G):
                    ln_chunk(NTB - 1, g, sh_tiles[NTB - 1], act_sq=True)
                for tb in range(NTB - 1):
                    f_block(tb)
                stats_block(NTB - 1, NTB)
                f_block(NTB - 1)


def declare_io(nc):
    return {
        "blob": nc.dram_tensor("blob", [_BLOB_BYTES], F8, kind="ExternalInput"),
        "yT": nc.dram_tensor("yT", [OUTD, STOK], FP, kind="ExternalOutput"),
    }


class _Sect:
    """dram-blob section handle mimicking a dram tensor (.ap() slicing)."""

    def __init__(self, blob, byte_off, n, shape, dt):
        self._blob, self._off, self._n = blob, byte_off, n
        self._shape, self._dt = shape, dt

    def ap(self):
        esz = {F8: 1, BF: 2, FP: 4}[self._dt]
        flat = self._blob.ap()[self._off : self._off + self._n * esz]
        if self._dt is not F8:
            flat = flat.bitcast(self._dt)
        dims = self._shape
        names = " ".join(f"d{i}" for i in range(len(dims)))
        kw = {f"d{i}": d for i, d in enumerate(dims)}
        return flat.rearrange(f"({names}) -> {names}", **kw)


def _sections(io):
    out = {}
    for name, (off, n, shape) in _F8_OFF.items():
        out[name] = _Sect(io["blob"], off, n, shape, F8)
    for name, (off, n, shape) in _BF_OFF.items():
        out[name] = _Sect(io["blob"], _BF_BASE + 2 * off, n, shape, BF)
    for name, (off, n, shape) in _F32_OFF.items():
        out[name] = _Sect(io["blob"], _F32_BASE + 4 * off, n, shape, FP)
    out["yT"] = io["yT"]
    return out


def build_nc():
    nc = bacc.Bacc("TRN2", target_bir_lowering=False, debug=False)
    io = declare_io(nc)
    with tile.TileContext(nc) as tc:
        _body(nc, tc, io)
    nc.compile()
    return nc


_CACHE = {}


def _get_nc():
    if "nc" not in _CACHE:
        _CACHE["nc"] = build_nc()
    return _CACHE["nc"]


def _block_w(w):
    """[cin=1024, cout] -> [cout_grp, p, j, b, cout_col] for DR lhsT tiles."""
    co = w.shape[1] // P
    return np.ascontiguousarray(
        w.reshape(4, 2, P, co, P).transpose(3, 2, 0, 1, 4)
    )


def make_in_maps(x, v, t, c, wq, bq, wkv, bkv, wproj, bproj, wada, bada, wlin, blin):
    f32 = lambda a: np.ascontiguousarray(np.asarray(a, dtype=np.float32))
    x, v, t, c = f32(x), f32(v), f32(t), f32(c)
    wq, wkv, wproj, wada = f32(wq), f32(wkv), f32(wproj), f32(wada)
    bq, bkv, bproj, bada = f32(bq), f32(bkv), f32(bproj), f32(bada)
    wlin, blin = f32(wlin), f32(blin)

    wkvv = np.ascontiguousarray(wkv[:, C:])
    wkvv8 = np.ascontiguousarray(
        (WS * wkvv).reshape(4, 2, P, 2, TB).transpose(3, 2, 0, 1, 4)
    ).astype(NP_F8)
    bkvv8 = np.zeros((1, 2, 2, TB), np.float32)
    bkvv8[0, :, 0, :] = (WS * bkv[C:]).reshape(2, TB)
    wada_b = np.ascontiguousarray(
        wada.reshape(G, P, 16, P).transpose(2, 1, 0, 3)
    ).astype(NP_BF)
    selc = np.zeros((P, 16, 2, 16), np.float32)
    for h in range(16):
        selc[:, h, :, h] = ONES_COL
    selp = np.zeros((16, G, 2, P), np.float32)
    for g in range(G):
        selp[2 * g, g, 0, 0:64] = 1.0
        selp[2 * g + 1, g, 0, 64:128] = 1.0

    f8_shared = {
        "wq8": _block_w(WS * wq).astype(NP_F8),
        "wkvk8": _block_w(WS * wkv[:, :C]).astype(NP_F8),
        "wkvv8": wkvv8.astype(NP_F8),
        "wproj8": _block_w(WS * wproj).astype(NP_F8),
        "bkvv8": bkvv8.astype(NP_F8),
        "selc8": selc.astype(NP_F8),
        "selp8": selp.astype(NP_F8),
    }
    bf_shared = {
        "wada_bf": wada_b,
        "wlin_bf": wlin.astype(NP_BF),
    }

    def pack(sects, parts, np_dt):
        flats = []
        for name, (off, n, shape) in sects.items():
            a = parts[name]
            assert a.size == n, (name, a.shape, shape)
            flats.append(np.ascontiguousarray(a).reshape(-1))
        return np.concatenate(flats).astype(np_dt, copy=False)

    in_maps = []
    vT8_cache = {}
    for m in range(8):
        b, half = divmod(m, 2)
        xs = x[b, half * STOK : (half + 1) * STOK, :]
        if b not in vT8_cache:
            vT8_cache[b] = np.ascontiguousarray(v[b].T).astype(NP_F8)
        f8_parts = {
            "xT8": np.ascontiguousarray(xs.T).astype(NP_F8),
            "vT8d": vT8_cache[b],
            **f8_shared,
        }
        bf_parts = {
            "xTbf": np.ascontiguousarray((xs + bproj[None, :]).T).astype(NP_BF),
            **bf_shared,
        }
        f32_parts = {
            "t_b": f32(t[b]),
            "c_sl": f32(c[b, half * F : (half + 1) * F, :]),
            "bq32": f32(WS * bq),
            "bkvk32": f32(WS * bkv[:C]),
            "bada": f32(bada),
            "blin": f32(blin),
        }
        blob = np.concatenate([
            pack(_F8_OFF, f8_parts, NP_F8).view(np.uint8),
            pack(_BF_OFF, bf_parts, NP_BF).view(np.uint8),
            pack(_F32_OFF, f32_parts, np.float32).view(np.uint8),
        ]).view(NP_F8)
        in_maps.append({"blob": blob})
    return in_maps


def assemble_y(results):
    y = np.empty((B, T, NT, OUTD), np.float32)
    for m in range(8):
        b, half = divmod(m, 2)
        yt = np.asarray(results[m]["yT"])  # [OUTD, STOK]
        y[b, half * F : (half + 1) * F] = yt.T.reshape(F, NT, OUTD)
    return y


def kernel(x, v, t, c, wq, bq, wkv, bkv, wproj, bproj, wada, bada, wlin, blin, T=16, H=16):
    nc = _get_nc()
    in_maps = make_in_maps(
        x, v, t, c, wq, bq, wkv, bkv, wproj, bproj, wada, bada, wlin, blin
    )
    res = run_bass_kernel_spmd(nc, in_maps, core_ids=list(range(8)))
    return assemble_y(res.results)
